# revision 41
# baseline (speedup 1.0000x reference)
"""Trainium2 Bass kernel for EnhancedTransformerBlock on ragged graphs.

Layout: transposed activations [channels (partitions), nodes (free)].
Sharding: 64 graphs -> 8 cores x 8 slots, assigned by size-sorted rank so
slot widths (uniform across cores, required for SPMD) hug the max count.

v2 design notes (vs the phase-batched f32r baseline):
- all matmul activations bf16; FFN + out_proj in fp8e4m3 with DoubleRow
  (contraction 256 per matmul at 0.5 cyc/row).
- scores: zero-padded per-head q replica (qZ) built once per slot with 4x-mode
  DVE copies; contraction 128 (4 heads of k x zero-trick).
- PV transposed: out [q<=128, 33] per head; col 33h+32 of vr holds 0.125 so the
  same matmul accumulates sumexp/8 (fp8 range prep for ctx).
- divide = stride-0 broadcast tensor_tensor; back-transpose on PE (identity).
- k needs no bias (cancels in softmax over keys); out_proj bias + wo@v_bias +
  ffn_b2 pre-added to x on host (GraphNorm is per-channel shift invariant).
- per-slot pipelining: attention(s) -> out_proj(s) -> gnorm2 stats(s); FFN per
  half interleaved with the other half's attention.
"""

import math
import numpy as np
import ml_dtypes

import concourse.bass as bass
import concourse.bacc as bacc
import concourse.mybir as mybir
import concourse.tile as tile
from concourse.bass_utils import run_bass_kernel_spmd
from contextlib import ExitStack

N_CORES = 8
B = 64
H = 256
NH = 8
HD = H // NH
EPS = 1e-5

F32 = mybir.dt.float32
BF16 = mybir.dt.bfloat16
FP8 = mybir.dt.float8e4
AF = mybir.ActivationFunctionType
OP = mybir.AluOpType
PM = mybir.MatmulPerfMode

NEG = -30.0          # additive key mask (pre-exp); exp(-30) == 0 in bf16
SC = 1.0 / math.sqrt(HD)
S1 = 32.0            # ffn_w1 fp8 prescale
S2 = 32.0            # ffn_w2 fp8 prescale
SO = 32.0            # out_proj_w fp8 prescale
SCX = 8.0            # ctx fp8 prescale (via 1/8 in the vr ones-column)


def _plan(batch):
    batch = np.asarray(batch).astype(np.int64)
    counts = np.bincount(batch, minlength=B).astype(np.int64)
    starts = np.concatenate([[0], np.cumsum(counts)[:-1]])
    order = np.argsort(-counts, kind="stable")  # rank -> graph id
    NS = B // N_CORES
    Ms, slot_graph = [], np.zeros((N_CORES, NS), np.int64)
    for s in range(NS):
        blk = order[N_CORES * s: N_CORES * s + N_CORES]
        m = int(max(16, math.ceil(max(1, counts[blk].max()) / 16) * 16))
        Ms.append(m)
        for c in range(N_CORES):
            slot_graph[c, s] = blk[c]
    offs = np.concatenate([[0], np.cumsum(Ms)]).astype(np.int64)
    Rtot = int(offs[-1])
    R = int(math.ceil(Rtot / 128) * 128)
    return counts, starts, slot_graph, Ms, offs, Rtot, R


def _build(nc, Ms, offs, R, ns_valid, pair_gelu):
    """ns_valid[s] = max valid node count over cores for slot s (<= Ms[s]).
    Per-core valid counts differ; we compute the slot at the max width and the
    km mask (per core) zeroes the prob rows beyond each core's own count.
    Query-side trims use ns_valid (same extent every core keeps SPMD single
    program); pads beyond ns_valid are never read back by any core."""
    NS = len(Ms)
    nkt = [math.ceil(m / 128) for m in Ms]
    NKT = sum(nkt)
    MMAX = max(Ms)

    # ---- DRAM ----
    d_xt = nc.dram_tensor("xt", [2, 128, R], BF16, kind="ExternalInput").ap()
    d_wqk = nc.dram_tensor("wqk", [128, 2, 512], BF16, kind="ExternalInput").ap()
    d_wvo = nc.dram_tensor("wvo", [128, 2, 264], BF16, kind="ExternalInput").ap()
    d_wo = nc.dram_tensor("wo", [128, 2, 2, 128], FP8, kind="ExternalInput").ap()
    d_w1 = nc.dram_tensor("w1", [128, 2, 1024], FP8, kind="ExternalInput").ap()
    d_w2 = nc.dram_tensor("w2", [128, 8, 2, 128], FP8, kind="ExternalInput").ap()
    # packed per-partition constants:
    # [qb(2) fb1(8) nb(4) ga1(NS) gAp(4*NS) gBp(4*NS) km(NKT)]
    NCST = 14 + 9 * NS + NKT
    d_cst = nc.dram_tensor("cst", [128, NCST], F32, kind="ExternalInput").ap()
    d_id = nc.dram_tensor("ident", [128, 128], BF16, kind="ExternalInput").ap()
    d_ot = nc.dram_tensor("ot", [2, 128, R], BF16, kind="ExternalOutput").ap()

    mm = nc.tensor.matmul

    with tile.TileContext(nc) as tc, ExitStack() as ctx:
        pers = ctx.enter_context(tc.tile_pool(name="pers", bufs=1))
        ptp = ctx.enter_context(tc.tile_pool(name="ptp", bufs=9))
        hgp = ctx.enter_context(tc.tile_pool(name="hgp", bufs=2))
        stat = ctx.enter_context(tc.tile_pool(name="stat", bufs=4))
        ctxp = ctx.enter_context(tc.tile_pool(name="ctxp", bufs=3))
        psP = ctx.enter_context(tc.tile_pool(name="psP", bufs=2, space="PSUM"))
        psS = ctx.enter_context(tc.tile_pool(name="psS", bufs=2, space="PSUM"))
        psC = ctx.enter_context(tc.tile_pool(name="psC", bufs=2, space="PSUM"))

        # ---- persistent SBUF tiles ----
        cst = pers.tile([128, NCST], F32, name="cst", tag="cst")
        nc.sync.dma_start(out=cst, in_=d_cst)
        co = 0
        def cslice(n):
            nonlocal co
            a = cst[:, co:co + n]; co += n
            return a
        qb = [cslice(1) for _ in range(2)]
        fb1 = [cslice(1) for _ in range(8)]
        nb = [[cslice(1) for _ in range(2)] for _ in range(2)]
        ga1 = cslice(NS)
        gAp = [[cslice(NS) for _ in range(2)] for _ in range(2)]
        gBp = [[cslice(NS) for _ in range(2)] for _ in range(2)]
        km = [cslice(1) for _ in range(NKT)]
        kmi = {}
        idx = 0
        for s in range(NS):
            for kt in range(nkt[s]):
                kmi[(s, kt)] = idx; idx += 1

        # Few LARGE DMAs, critical-first: each DMACopy pays ~1.4us of fixed
        # HWDGE/sem overhead, so slot-granular loads serialize the startup.
        q_off = int(offs[NS // 4])
        half_off = int(offs[NS // 2])
        ident = pers.tile([128, 128], BF16, name="ident", tag="ident")
        nc.sync.dma_start(out=ident, in_=d_id)
        xt = [pers.tile([128, R], BF16, name=f"xt{i}", tag=f"xt{i}") for i in range(2)]
        for ct in range(2):
            nc.sync.dma_start(out=xt[ct][:, 0:q_off], in_=d_xt[ct][:, 0:q_off])
        wqkt = pers.tile([128, 2, 512], BF16, name="wqkt", tag="wqkt")
        nc.sync.dma_start(out=wqkt, in_=d_wqk)
        wvot = pers.tile([128, 2, 264], BF16, name="wvot", tag="wvot")
        nc.sync.dma_start(out=wvot, in_=d_wvo)
        for ct in range(2):
            nc.sync.dma_start(out=xt[ct][:, q_off:half_off], in_=d_xt[ct][:, q_off:half_off])
        for ct in range(2):
            nc.sync.dma_start(out=xt[ct][:, half_off:R], in_=d_xt[ct][:, half_off:R])
        wot = pers.tile([128, 2, 2, 128], FP8, name="wot", tag="wot")
        nc.sync.dma_start(out=wot, in_=d_wo)
        w1 = pers.tile([128, 2, 1024], FP8, name="w1", tag="w1")
        nc.sync.dma_start(out=w1, in_=d_w1)
        w2t = pers.tile([128, 8, 2, 128], FP8, name="w2t", tag="w2t")
        nc.sync.dma_start(out=w2t, in_=d_w2)

        # PE p-state warmup: ~24 dep-free matmuls keep the PE continuously
        # busy from t~1us so real matmuls start at the fast clock.
        for _ in range(36):
            wps = psP.tile([128, 512], F32, name="psp", tag="psp")
            mm(wps[:, :128], ident, ident, start=True, stop=True)

        xn = [pers.tile([128, R], BF16, name=f"xn{i}", tag=f"xn{i}") for i in range(2)]
        qt_ = [pers.tile([128, R], BF16, name=f"q{i}", tag=f"q{i}") for i in range(2)]
        kt_ = [pers.tile([128, R], BF16, name=f"k{i}", tag=f"k{i}") for i in range(2)]
        qZ = pers.tile([128, 8, MMAX], BF16, name="qZ", tag="qZ")
        nc.gpsimd.memset(qZ, 0.0)  # persistent zeros; head h only ever writes rows 32*(h%4)
        vr = pers.tile([128, 264 * NKT], BF16, name="vr", tag="vr")
        ctxt = pers.tile([128, 2, R], FP8, name="ctxt", tag="ctxt")
        x2 = [pers.tile([128, R], BF16, name=f"x2{i}", tag=f"x2{i}") for i in range(2)]
        for ct in range(2):
            nc.gpsimd.memset(x2[ct], 0.0)  # pads must stay 0 for gnorm2 stats
        xn2 = pers.tile([128, 2, R], FP8, name="xn2", tag="xn2")
        out_t = [pers.tile([128, R], BF16, name=f"ot{i}", tag=f"ot{i}") for i in range(2)]

        # ---------- GraphNorm stats+apply ----------
        # rstd = rsqrt(var) via DVE reciprocal seed + Newton (keeps the ACT
        # engine free of Ln/Exp and their 1283ns table loads; eps ~ 1e-5 on a
        # ~1.0 std is far below the fp8/bf16 noise floor, dropped).
        def gnorm_stats(src_f, widx, slots, tg):
            """bn_stats (DVE only) + var/mean prep; returns (var, meanp)."""
            nsl = len(slots)
            c0 = slots[0]
            var = stat.tile([128, 2, nsl], F32, name="var", tag=f"var{tg}")
            meanp = stat.tile([128, 2, nsl], F32, name="meanp", tag=f"meanp{tg}")
            for ct in range(2):
                mv = stat.tile([128, 2, nsl], F32, name="mv", tag=f"mv{tg}{ct}")
                for i, s in enumerate(slots):
                    st6 = stat.tile([128, 6], F32, name="st6", tag="st6")
                    nc.vector.bn_stats(out=st6, in_=src_f(ct)[:, offs[s]:offs[s] + Ms[s]])
                    nc.vector.bn_aggr(out=mv[:, :, i:i + 1], in_=st6)
                mean_r = mv[:, 0:1, :].squeeze(1)
                var_r = mv[:, 1:2, :].squeeze(1)
                m2 = stat.tile([128, nsl], F32, name="m2", tag="m2")
                nc.vector.tensor_mul(m2, mean_r, mean_r)
                v1 = stat.tile([128, nsl], F32, name="v1", tag="v1")
                nc.vector.tensor_mul(v1, var_r, gAp[widx][ct][:, c0:c0 + nsl])
                nc.vector.tensor_mul(var[:, ct, :], m2, gBp[widx][ct][:, c0:c0 + nsl])
                nc.vector.tensor_add(var[:, ct, :], var[:, ct, :], v1)
                nc.vector.tensor_mul(meanp[:, ct, :], mean_r, ga1[:, c0:c0 + nsl])
            return var, meanp

        def gnorm_chain(var, meanp, widx, slots, tg, eng, recip_seed):
            """rsqrt chain -> (y == scale, per-ct shift). Runs on `eng` so the
            ~6us of serial hop latency doesn't head-of-line block DVE."""
            nsl = len(slots)
            vv = var[:, :, :]
            y = stat.tile([128, 2, nsl], F32, name="y", tag=f"y{tg}")
            t = stat.tile([128, 2, nsl], F32, name="t", tag=f"t{tg}")
            if recip_seed:
                nc.vector.reciprocal_approx_fast(out=y, in_=vv)
                eng.tensor_scalar(out=y, in0=y, scalar1=0.5, scalar2=0.5,
                                  op0=OP.mult, op1=OP.add)
                iters = 1
            else:  # linear seed 1.5 - 0.5v (fine for var in [0.5, 1.6])
                eng.tensor_scalar(out=y, in0=vv, scalar1=-0.5, scalar2=1.5,
                                  op0=OP.mult, op1=OP.add)
                iters = 2
            for _ in range(iters):
                eng.tensor_mul(t, vv, y)
                eng.tensor_mul(t, t, y)
                eng.tensor_scalar(out=t, in0=t, scalar1=-0.5, scalar2=1.5,
                                  op0=OP.mult, op1=OP.add)
                eng.tensor_mul(y, y, t)
            sc_sh = []
            for ct in range(2):
                shift = stat.tile([128, nsl], F32, name="shift", tag=f"shift{tg}{ct}")
                eng.tensor_mul(shift, meanp[:, ct, :], y[:, ct, :])
                eng.tensor_scalar(
                    out=shift, in0=shift, scalar1=-1.0, scalar2=nb[widx][ct],
                    op0=OP.mult, op1=OP.add)
                sc_sh.append(shift)
            return y, sc_sh

        def gnorm_apply(src_f, dst_f, y, sc_sh, slots, awidths, eng, sel=None):
            for i, s in enumerate(slots):
                if sel is not None and s not in sel:
                    continue
                for ct in range(2):
                    w = awidths[s]
                    if eng is nc.scalar:  # ACT: out = Identity(scale*in + bias)
                        nc.scalar.activation(
                            out=dst_f(ct, s, w),
                            in_=src_f(ct)[:, offs[s]:offs[s] + w],
                            func=AF.Identity,
                            bias=sc_sh[ct][:, i:i + 1],
                            scale=y[:, ct, i:i + 1])
                    else:
                        eng.tensor_scalar(
                            out=dst_f(ct, s, w),
                            in0=src_f(ct)[:, offs[s]:offs[s] + w],
                            scalar1=y[:, ct, i:i + 1],
                            scalar2=sc_sh[ct][:, i:i + 1],
                            op0=OP.mult, op1=OP.add)

        def gnorm(src_f, dst_f, widx, slots, awidths, tg,
                  chain_eng=None, apply_eng=None, recip_seed=True):
            var, meanp = gnorm_stats(src_f, widx, slots, tg)
            y, sc_sh = gnorm_chain(var, meanp, widx, slots, tg,
                                   chain_eng or nc.vector, recip_seed)
            gnorm_apply(src_f, dst_f, y, sc_sh, slots, awidths,
                        apply_eng or (nc.vector if widx == 0 else nc.gpsimd))
            return y, sc_sh



        # ---------- phase 2: q,k projections (slot-aligned chunks so each
        # slot's qZ depends only on its own gnorm pair; pads never projected) --
        def qkproj_chunk(o, w, on_act=False):
            for mt in range(4):
                ps = psP.tile([128, 512], F32, name="psp", tag="psp")
                for ktc in range(2):
                    mm(ps[:, :w], wqkt[:, ktc, 128 * mt:128 * mt + 128],
                       xn[ktc][:, o:o + w], start=(ktc == 0), stop=(ktc == 1))
                if mt < 2:  # q: add bias (k bias cancels in softmax)
                    if on_act:
                        nc.scalar.activation(out=qt_[mt][:, o:o + w], in_=ps[:, :w],
                                             func=AF.Identity, bias=qb[mt])
                    else:
                        nc.vector.tensor_scalar_add(qt_[mt][:, o:o + w], ps[:, :w], qb[mt])
                else:
                    if on_act:
                        nc.scalar.activation(out=kt_[mt - 2][:, o:o + w], in_=ps[:, :w],
                                             func=AF.Copy)
                    else:
                        nc.vector.tensor_copy(kt_[mt - 2][:, o:o + w], ps[:, :w])

        def qkproj_slot(s, on_act=False):
            qkproj_chunk(int(offs[s]), int(Ms[s]), on_act)

        vri = {}
        idx = 0
        for s in range(NS):
            for kt in range(nkt[s]):
                vri[(s, kt)] = idx; idx += 1
        def vproj(s, kt, on_act=False):
            mkt = min(128, Ms[s] - 128 * kt)
            ko = offs[s] + 128 * kt
            vb = 264 * vri[(s, kt)]
            ps = psP.tile([128, 512], F32, name="psp", tag="psp")
            for ctc in range(2):
                mm(ps[:mkt, :264], xn[ctc][:, ko:ko + mkt], wvot[:, ctc],
                   start=(ctc == 0), stop=(ctc == 1))
            if on_act:
                nc.scalar.activation(out=vr[:mkt, vb:vb + 264], in_=ps[:mkt, :264],
                                     func=AF.Copy)
            else:
                nc.vector.tensor_copy(vr[:mkt, vb:vb + 264], ps[:mkt, :264])
            # sumexp ones-column = 1/SCX (ctx fp8 prescale rides the ratio)
            ones = vr[:mkt, vb:vb + 264].rearrange("p (h c) -> p h c", h=8, c=33)[:, :, 32:33].squeeze(2)
            nc.gpsimd.memset(ones, 1.0 / SCX)

        # ---------- attention, software-pipelined over (slot, qtile) ----------
        def qz_slot(s, on_dve):
            M = Ms[s]
            eng = nc.vector if on_dve else nc.gpsimd
            for h in range(8):
                hp = 32 * (h % 4)
                eng.tensor_copy(qZ[hp:hp + 32, h, :M],
                                qt_[h // 4][hp:hp + 32, offs[s]:offs[s] + M])

        def attn_front(s, qi):
            """scores + exp for all key tiles of (s, qi); returns state."""
            M = Ms[s]; nv = ns_valid[s]
            qo = 128 * qi
            qc = min(128, nv - qo)
            pts = []
            for kt in range(nkt[s]):
                mkt = min(128, M - 128 * kt)
                ko = offs[s] + 128 * kt
                st = psS.tile([128, 1024], F32, name="st", tag="st")
                for h in range(8):
                    mm(st[:mkt, 128 * h:128 * h + qc],
                       kt_[h // 4][:, ko:ko + mkt], qZ[:, h, qo:qo + qc],
                       start=True, stop=True)
                pt = ptp.tile([128, 1024], BF16, name="pt", tag="pt")
                stv = st[:mkt, :].rearrange("p (h c) -> p h c", h=8, c=128)[:, :, :qc]
                ptv = pt[:mkt, :].rearrange("p (h c) -> p h c", h=8, c=128)[:, :, :qc]
                nc.scalar.activation(out=ptv, in_=stv, func=AF.Exp,
                                     bias=km[kmi[(s, kt)]][:mkt], scale=SC)
                pts.append(pt)
            return pts

        def attn_back(s, qi, pts):
            """PV + divide + transpose + ctxt store for (s, qi)."""
            M = Ms[s]; nv = ns_valid[s]
            qo = 128 * qi
            qc = min(128, nv - qo)
            qbase = offs[s] + qo
            # cs ([128,264] f32) + tp ([128,2,128] bf16 via bitcast) share
            # one PSUM bank: 1056B + 512B < 2KB
            csbank = psC.tile([128, 392], F32, name="csbank", tag="cs")
            cs = csbank[:, 0:264]
            tp = csbank[:, 264:392].bitcast(BF16).rearrange(
                "p (t c) -> p t c", t=2, c=128)
            for kt in range(nkt[s]):
                mkt = min(128, M - 128 * kt)
                vb = 264 * vri[(s, kt)]
                last = kt == nkt[s] - 1
                for h in range(8):
                    mm(cs[:qc, 33 * h:33 * h + 33],
                       pts[kt][:mkt, 128 * h:128 * h + qc],
                       vr[:mkt, vb + 33 * h:vb + 33 * h + 33],
                       start=(kt == 0), stop=last)
            # rec = SCX / sumexp ; ctxT = cs * rec (broadcast over 33-blocks)
            rec = stat.tile([128, 8], F32, name="rec", tag="rec")
            den = cs[:qc, :].rearrange("p (h c) -> p h c", h=8, c=33)[:, :, 32:33].squeeze(2)
            nc.vector.reciprocal_approx_fast(out=rec[:qc, :], in_=den)
            ctxTs = ctxp.tile([128, 256], BF16, name="ctxTs", tag="ctxTs")
            csv = cs[:qc, :].rearrange("p (h c) -> p h c", h=8, c=33)[:, :, 0:32]
            ctv = ctxTs[:qc, :].rearrange("p (h c) -> p h c", h=8, c=32)
            rv = rec[:qc, :].unsqueeze(2).broadcast_to([qc, 8, 32])
            nc.vector.tensor_tensor(out=ctv, in0=csv, in1=rv, op=OP.mult)
            for ctc in range(2):
                nc.tensor.transpose(tp[:, ctc, :qc], ctxTs[:qc, 128 * ctc:128 * ctc + 128],
                                    ident[:qc, :qc])
            nc.vector.tensor_copy(ctxt[:, :, qbase:qbase + qc], tp[:, :, :qc])

        # ---------- out_proj + residual (valid width only; pads stay 0) ----
        def outproj_slot(s):
            nv = ns_valid[s]
            o = offs[s]
            for ctc in range(2):
                ps = psP.tile([128, 512], F32, name="psp", tag="psp")
                for cw in range(0, nv, 256):  # DoubleRow rhs free = 2*w <= 512
                    w = min(256, nv - cw)
                    mm(ps[:, cw:cw + w], wot[:, ctc], ctxt[:, :, o + cw:o + cw + w],
                       start=True, stop=True, perf_mode=PM.DoubleRow)
                nc.vector.scalar_tensor_tensor(
                    out=x2[ctc][:, o:o + nv], in0=ps[:, :nv], scalar=1.0 / (SCX * SO),
                    in1=xt[ctc][:, o:o + nv], op0=OP.mult, op1=OP.add)

        # ---------- FFN, two stages for slot-level staggering ----------
        def ffn1_slot(s, pair_gelu):
            nv = ns_valid[s]
            o = offs[s]
            hg = [hgp.tile([128, 2, MMAX], FP8, name=f"hg{p}", tag=f"hg{p}") for p in range(4)]
            for p in range(4):
                if pair_gelu:
                    # one [128,1024] psum holds the mt-pair; one Gelu covers both
                    ps = psS.tile([128, 1024], F32, name="st", tag="st")
                    for half in range(2):
                        mt = 2 * p + half
                        for cw in range(0, nv, 256):
                            w = min(256, nv - cw)
                            mm(ps[:, 512 * half + cw:512 * half + cw + w],
                               w1[:, :, 128 * mt:128 * mt + 128],
                               xn2[:, :, o + cw:o + cw + w],
                               start=True, stop=True, perf_mode=PM.DoubleRow)
                    psv = ps[:, :].rearrange("p (t c) -> p t c", t=2, c=512)[:, :, :nv]
                    nc.scalar.activation(out=hg[p][:, :, :nv], in_=psv,
                                         func=AF.Gelu, scale=1.0 / S1)
                else:
                    for half in range(2):
                        mt = 2 * p + half
                        ps = psP.tile([128, 512], F32, name="psp", tag="psp")
                        for cw in range(0, nv, 256):
                            w = min(256, nv - cw)
                            mm(ps[:, cw:cw + w], w1[:, :, 128 * mt:128 * mt + 128],
                               xn2[:, :, o + cw:o + cw + w],
                               start=True, stop=True, perf_mode=PM.DoubleRow)
                        nc.scalar.activation(out=hg[p][:, half, :nv], in_=ps[:, :nv],
                                             func=AF.Gelu, bias=fb1[mt], scale=1.0 / S1)
            return hg

        def ffn2_slot(s, hg):
            nv = ns_valid[s]
            o = offs[s]
            for ctc in range(2):
                ps2 = psP.tile([128, 512], F32, name="psp", tag="psp")
                for cw in range(0, nv, 256):
                    w = min(256, nv - cw)
                    for p in range(4):
                        mm(ps2[:, cw:cw + w], w2t[:, 4 * ctc + p], hg[p][:, :, cw:cw + w],
                           start=(p == 0), stop=(p == 3), perf_mode=PM.DoubleRow)
                nc.vector.scalar_tensor_tensor(
                    out=out_t[ctc][:, o:o + nv], in0=ps2[:, :nv], scalar=1.0 / S2,
                    in1=x2[ctc][:, o:o + nv], op0=OP.mult, op1=OP.add)

        # ---------- main pipelined emission ----------
        # attention staggered by one (s, qi) item: scores/exp of item i+1 sit
        # ahead of PV(i) in the PE queue, so PE never head-of-line blocks on
        # an exp. FFN staggered by one slot for the same reason. ACT stream
        # stays [exps..., gelus...] to avoid 1283ns table reloads.
        # startup: absolute minimum before the first scores/exp of slot 0 —
        # gnorm1 pair {s0,s1}, projections of s0/s1, qZ(0), vproj(0). The
        # other pairs are emitted inside the first loop iterations and
        # pipeline their (stats -> rsqrt chain -> apply -> proj) latency
        # behind the running exp stream.
        xn_dst = lambda ct, s, w: xn[ct][:, offs[s]:offs[s] + w]
        xt_src = lambda ct: xt[ct]
        gnorm(xt_src, xn_dst, 0, [0, 1], {s: Ms[s] for s in range(NS)}, "g1p0")
        qkproj_slot(0, on_act=True)
        qz_slot(0, True)
        qkproj_slot(1, on_act=True)
        for kt in range(nkt[0]):
            vproj(0, kt, on_act=True)
        qz_slot(1, True)
        for kt in range(nkt[1]):
            vproj(1, kt, on_act=True)
        # slots 2-7: stats early on DVE; the serial rsqrt chain runs on Pool
        # so its latency can't head-of-line block DVE's divide stream; applies
        # + projection drains go to ACT *between* the pair boundaries of the
        # exp stream (they fill what would otherwise be ramp gaps).
        g1v, g1m = gnorm_stats(xt_src, 0, [2, 3, 4, 5, 6, 7], "g1r")
        g1y, g1s = gnorm_chain(g1v, g1m, 0, [2, 3, 4, 5, 6, 7], "g1r",
                               nc.gpsimd, recip_seed=False)

        def deferred_startup(i):
            if i in (2, 5, 8):
                p = i // 3 + 1
                gnorm_apply(xt_src, xn_dst, g1y, g1s, [2, 3, 4, 5, 6, 7],
                            {s: Ms[s] for s in range(NS)}, nc.scalar,
                            sel={2 * p, 2 * p + 1})
                qkproj_slot(2 * p, on_act=True)
                qkproj_slot(2 * p + 1, on_act=True)

        items = [(s, qi) for s in range(NS) for qi in range(math.ceil(ns_valid[s] / 128))]
        DEPTH = 2
        fronts = {}
        g2_half = {}

        def do_back(j):
            sj, qj = items[j]
            attn_back(sj, qj, fronts.pop(j))
            if qj == math.ceil(ns_valid[sj] / 128) - 1:
                outproj_slot(sj)
                if sj == 3:
                    g2_half[0] = gnorm(
                        lambda ct: x2[ct],
                        lambda ct, s2, w: xn2[:, ct, offs[s2]:offs[s2] + w],
                        1, [0, 1, 2, 3], {t: ns_valid[t] for t in range(NS)}, "g2a",
                        chain_eng=nc.gpsimd, recip_seed=False)

        for i, (s, qi) in enumerate(items):
            deferred_startup(i)
            if qi == math.ceil(ns_valid[s] / 128) - 1 and s + 2 < NS:
                qz_slot(s + 2, False)
                for kt in range(nkt[s + 2]):
                    vproj(s + 2, kt)
            fronts[i] = attn_front(s, qi)
            if i >= DEPTH:
                do_back(i - DEPTH)
        for j in range(len(items) - DEPTH, len(items)):
            do_back(j)
        gnorm(lambda ct: x2[ct],
              lambda ct, s2, w: xn2[:, ct, offs[s2]:offs[s2] + w],
              1, [4, 5, 6, 7], {t: ns_valid[t] for t in range(NS)}, "g2b",
              chain_eng=nc.gpsimd, recip_seed=False)
        hgprev = None
        for s in range(NS):
            hgnew = ffn1_slot(s, pair_gelu)
            if hgprev is not None:
                ffn2_slot(s - 1, hgprev)
            hgprev = hgnew
            if s == NS // 2:
                for ctc in range(2):
                    nc.sync.dma_start(out=d_ot[ctc][:, 0:offs[NS // 2]],
                                      in_=out_t[ctc][:, 0:offs[NS // 2]])
        ffn2_slot(NS - 1, hgprev)
        for ctc in range(2):
            nc.sync.dma_start(out=d_ot[ctc][:, offs[NS // 2]:R],
                              in_=out_t[ctc][:, offs[NS // 2]:R])
    return nc


_CACHE = {}


def _prepare(inputs):
    x = np.asarray(inputs["x"], np.float32)
    batch = np.asarray(inputs["batch"]).astype(np.int64)
    counts, starts, slot_graph, Ms, offs, Rtot, R = _plan(batch)
    NS = len(Ms)
    nkt = [math.ceil(m / 128) for m in Ms]
    NKT = sum(nkt)

    in_proj_w = np.asarray(inputs["in_proj_w"], np.float32)
    in_proj_b = np.asarray(inputs["in_proj_b"], np.float32)
    out_proj_w = np.asarray(inputs["out_proj_w"], np.float32)
    out_proj_b = np.asarray(inputs["out_proj_b"], np.float32)
    ffn_w1 = np.asarray(inputs["ffn_w1"], np.float32)
    ffn_b1 = np.asarray(inputs["ffn_b1"], np.float32)
    ffn_w2 = np.asarray(inputs["ffn_w2"], np.float32)
    ffn_b2 = np.asarray(inputs["ffn_b2"], np.float32)

    # biases folded into the residual stream (gnorm is shift-invariant):
    # x' = x + out_proj_b + wo @ v_bias + ffn_b2
    fold = out_proj_b + out_proj_w @ in_proj_b[2 * H:3 * H] + ffn_b2
    xb = x + fold[None, :]

    wqk = np.ascontiguousarray(
        in_proj_w[:2 * H].T.reshape(2, 128, 512).transpose(1, 0, 2)).astype(ml_dtypes.bfloat16)
    # wv expanded to 33-col stride with zero ones-columns
    wvT = in_proj_w[2 * H:].T.reshape(2, 128, 8, 32)
    wvo = np.zeros((2, 128, 8, 33), np.float32)
    wvo[:, :, :, :32] = wvT
    wvo = np.ascontiguousarray(
        wvo.reshape(2, 128, 264).transpose(1, 0, 2)).astype(ml_dtypes.bfloat16)
    # wo fp8 [ct_out][128, 2(plane=ct_in), 128], prescaled
    woT = (out_proj_w.T * SO).reshape(2, 128, 2, 128)   # [ct_in, part, ct_out, col]
    wo8 = np.ascontiguousarray(woT.transpose(1, 2, 0, 3)).astype(ml_dtypes.float8_e4m3)
    # w1 fp8 [128, 2, 1024]: plane = input ct
    w18 = np.ascontiguousarray((ffn_w1.T * S1).reshape(2, 128, 1024).transpose(1, 0, 2)).astype(ml_dtypes.float8_e4m3)
    # w2 fp8 [8][128, 2, 128]: idx = 4*ct_out + pair; plane i = hidden 256p+128i
    w2T = (ffn_w2.T * S2).reshape(4, 2, 128, 2, 128)    # [pair, plane, part, ct_out, col]
    # [part, idx=4*ct_out+pair, plane, col]
    w28 = np.ascontiguousarray(
        w2T.transpose(2, 3, 0, 1, 4).reshape(128, 8, 2, 128)).astype(ml_dtypes.float8_e4m3)

    qkb = in_proj_b[:2 * H].reshape(4, 128)
    fb1 = ffn_b1.reshape(8, 128)
    nwv = np.stack([np.asarray(inputs["norm1_w"], np.float32).reshape(2, 128),
                    np.asarray(inputs["norm2_w"], np.float32).reshape(2, 128)])
    nbv = np.stack([np.asarray(inputs["norm1_b"], np.float32).reshape(2, 128),
                    np.asarray(inputs["norm2_b"], np.float32).reshape(2, 128)])

    xT = xb.T  # [256, N]
    xts = np.zeros((N_CORES, 2, 128, R), np.float32)
    ga1 = np.zeros((N_CORES, 128, NS), np.float32)
    gA = np.zeros((N_CORES, 128, NS), np.float32)
    gB = np.zeros((N_CORES, 128, NS), np.float32)
    kms = np.full((N_CORES, NKT, 128), NEG, np.float32)
    ns_valid = [0] * NS
    for c in range(N_CORES):
        for s in range(NS):
            g = slot_graph[c, s]
            n = int(counts[g])
            st = int(starts[g])
            o = int(offs[s])
            ns_valid[s] = max(ns_valid[s], n)
            if n > 0:
                blk = xT[:, st:st + n]
                xts[c, 0, :, o:o + n] = blk[:128]
                xts[c, 1, :, o:o + n] = blk[128:]
            ne = max(n, 1)
            ga1[c, :, s] = Ms[s] / ne
            inv_nm1 = 1.0 / max(ne - 1, 1)
            gA[c, :, s] = Ms[s] * inv_nm1
            gB[c, :, s] = Ms[s] * (1.0 - Ms[s] / ne) * inv_nm1
            ki = sum(nkt[:s])
            for kt in range(nkt[s]):
                v = min(128, max(0, n - 128 * kt))
                kms[c, ki + kt, :v] = 0.0
    ns_valid = [int(math.ceil(v / 16) * 16) if v % 16 else v for v in ns_valid]
    ns_valid = [min(v, Ms[s]) for s, v in enumerate(ns_valid)]

    pair_gelu = bool(np.all(ffn_b1 == 0))
    key = (tuple(Ms), R, tuple(ns_valid), pair_gelu)
    if key not in _CACHE:
        nc = bacc.Bacc("TRN2", target_bir_lowering=False, debug=False,
                       num_devices=N_CORES)
        _build(nc, Ms, offs, R, ns_valid, pair_gelu)
        nc.compile()
        _CACHE[key] = nc
    nc = _CACHE[key]

    ident = np.eye(128).astype(ml_dtypes.bfloat16)
    in_maps = []
    for c in range(N_CORES):
        # norm_w folded: y = rsqrt(var / nw^2) = nw * rsqrt(var) (nw >= 0)
        nw2 = np.maximum(nwv.reshape(2, 2, 128), 1e-20) ** 2  # [phase, ct, 128]
        gparts = []
        for gsrc in (gA[c], gB[c]):           # [128, NS]
            for ph in range(2):
                for ctc in range(2):
                    gparts.append(gsrc / nw2[ph, ctc][:, None])
        cstc = np.concatenate(
            [qkb[:2].T,                       # qb (q bias only)
             fb1.T,                           # 8
             nbv.reshape(4, 128).T,           # 4
             ga1[c]] + gparts +               # NS + 8*NS
            [kms[c].T], axis=1).astype(np.float32)
        in_maps.append({
            "xt": xts[c].astype(ml_dtypes.bfloat16),
            "wqk": wqk, "wvo": wvo, "wo": wo8, "w1": w18, "w2": w28,
            "cst": np.ascontiguousarray(cstc),
            "ident": ident,
        })

    def unpack(outs):
        out = np.empty((x.shape[0], H), np.float32)
        for c in range(N_CORES):
            ot = np.asarray(outs[c]["ot"]).astype(np.float32)  # [2, 128, R]
            full = np.concatenate([ot[0], ot[1]], axis=0)      # [256, R]
            for s in range(NS):
                g = slot_graph[c, s]
                n = int(counts[g])
                st = int(starts[g])
                o = int(offs[s])
                if n > 0:
                    out[st:st + n] = full[:, o:o + n].T
        return out

    return nc, in_maps, unpack


def kernel(**inputs):
    nc, in_maps, unpack = _prepare(inputs)
    res = run_bass_kernel_spmd(nc, in_maps, list(range(N_CORES)))
    return unpack(res.results)


def _traced_run(**inputs):
    """Cost-model timeline (single core) + warm wall-clock. Returns model ns."""
    import time
    nc, in_maps, unpack = _prepare(inputs)
    t0 = time.time()
    run_bass_kernel_spmd(nc, in_maps, list(range(N_CORES)))
    t1 = time.time()
    run_bass_kernel_spmd(nc, in_maps, list(range(N_CORES)))
    t2 = time.time()
    print(f"wall cold: {t1 - t0:.2f}s  warm: {t2 - t1:.2f}s")
    from concourse.timeline_sim import TimelineSim
    import trails.perfetto as _tp
    for _m in ("enable_explicit_ordering", "reserve_process_order",
               "reserve_thread_order", "set_process_order", "set_thread_order",
               "add_instant"):
        if not hasattr(_tp.LazyPerfetto, _m):
            setattr(_tp.LazyPerfetto, _m, lambda self, *a, **k: None)
    if not hasattr(_tp.LazyPerfetto, "add_counter"):
        def _add_counter(self, *a, **k):
            try:
                self.update_counter(*a, **k)
            except Exception:
                pass
        _tp.LazyPerfetto.add_counter = _add_counter
    tl = TimelineSim(nc, trace=True)
    total = tl.simulate()
    pf = tl.perfetto
    if callable(pf):
        pf = pf()
    if pf is not None:
        try:
            pf.save("/root/problem/tl.perfetto-trace")
        except Exception as e:
            print("perfetto dump failed:", e)
    return total


# revision 43
# speedup vs baseline: 1.1145x; 1.1145x over previous
"""Trainium2 Bass kernel for EnhancedTransformerBlock on ragged graphs.

Layout: transposed activations [channels (partitions), nodes (free)].
Sharding: 64 graphs -> 8 cores x 8 slots, assigned by size-sorted rank so
slot widths (uniform across cores, required for SPMD) hug the max count.

v2 design notes (vs the phase-batched f32r baseline):
- all matmul activations bf16; FFN + out_proj in fp8e4m3 with DoubleRow
  (contraction 256 per matmul at 0.5 cyc/row).
- scores: zero-padded per-head q replica (qZ) built once per slot with 4x-mode
  DVE copies; contraction 128 (4 heads of k x zero-trick).
- PV transposed: out [q<=128, 33] per head; col 33h+32 of vr holds 0.125 so the
  same matmul accumulates sumexp/8 (fp8 range prep for ctx).
- divide = stride-0 broadcast tensor_tensor; back-transpose on PE (identity).
- k needs no bias (cancels in softmax over keys); out_proj bias + wo@v_bias +
  ffn_b2 pre-added to x on host (GraphNorm is per-channel shift invariant).
- per-slot pipelining: attention(s) -> out_proj(s) -> gnorm2 stats(s); FFN per
  half interleaved with the other half's attention.
"""

import math
import numpy as np
import ml_dtypes

import concourse.bass as bass
import concourse.bacc as bacc
import concourse.mybir as mybir
import concourse.tile as tile
from concourse.bass_utils import run_bass_kernel_spmd
from contextlib import ExitStack

N_CORES = 8
B = 64
H = 256
NH = 8
HD = H // NH
EPS = 1e-5

F32 = mybir.dt.float32
BF16 = mybir.dt.bfloat16
FP8 = mybir.dt.float8e4
AF = mybir.ActivationFunctionType
OP = mybir.AluOpType
PM = mybir.MatmulPerfMode

NEG = -30.0          # additive key mask (pre-exp); exp(-30) == 0 in bf16
SC = 1.0 / math.sqrt(HD)
S1 = 32.0            # ffn_w1 fp8 prescale
S2 = 32.0            # ffn_w2 fp8 prescale
SO = 32.0            # out_proj_w fp8 prescale
SCX = 8.0            # ctx fp8 prescale (via 1/8 in the vr ones-column)


def _plan(batch):
    batch = np.asarray(batch).astype(np.int64)
    counts = np.bincount(batch, minlength=B).astype(np.int64)
    starts = np.concatenate([[0], np.cumsum(counts)[:-1]])
    order = np.argsort(-counts, kind="stable")  # rank -> graph id
    NS = B // N_CORES
    Ms, slot_graph = [], np.zeros((N_CORES, NS), np.int64)
    for s in range(NS):
        blk = order[N_CORES * s: N_CORES * s + N_CORES]
        m = int(max(16, math.ceil(max(1, counts[blk].max()) / 16) * 16))
        Ms.append(m)
        for c in range(N_CORES):
            slot_graph[c, s] = blk[c]
    offs = np.concatenate([[0], np.cumsum(Ms)]).astype(np.int64)
    Rtot = int(offs[-1])
    R = int(math.ceil(Rtot / 128) * 128)
    return counts, starts, slot_graph, Ms, offs, Rtot, R


def _build(nc, Ms, offs, R, ns_valid, pair_gelu):
    """ns_valid[s] = max valid node count over cores for slot s (<= Ms[s]).
    Per-core valid counts differ; we compute the slot at the max width and the
    km mask (per core) zeroes the prob rows beyond each core's own count.
    Query-side trims use ns_valid (same extent every core keeps SPMD single
    program); pads beyond ns_valid are never read back by any core."""
    NS = len(Ms)
    nkt = [math.ceil(m / 128) for m in Ms]
    NKT = sum(nkt)
    MMAX = max(Ms)

    # ---- DRAM ----
    d_xt = nc.dram_tensor("xt", [2, 128, R], BF16, kind="ExternalInput").ap()
    d_wqk = nc.dram_tensor("wqk", [128, 2, 512], BF16, kind="ExternalInput").ap()
    d_wvo = nc.dram_tensor("wvo", [128, 2, 264], BF16, kind="ExternalInput").ap()
    d_wo = nc.dram_tensor("wo", [128, 2, 2, 128], FP8, kind="ExternalInput").ap()
    d_w1 = nc.dram_tensor("w1", [128, 2, 1024], FP8, kind="ExternalInput").ap()
    d_w2 = nc.dram_tensor("w2", [128, 8, 2, 128], FP8, kind="ExternalInput").ap()
    # packed per-partition constants:
    # [qb(2) fb1(8) nb(4) ga1(NS) gAp(4*NS) gBp(4*NS) km(NKT)]
    NCST = 14 + 9 * NS + NKT
    d_cst = nc.dram_tensor("cst", [128, NCST], F32, kind="ExternalInput").ap()
    d_id = nc.dram_tensor("ident", [128, 128], BF16, kind="ExternalInput").ap()
    d_ot = nc.dram_tensor("ot", [2, 128, R], BF16, kind="ExternalOutput").ap()

    mm = nc.tensor.matmul

    with tile.TileContext(nc) as tc, ExitStack() as ctx:
        pers = ctx.enter_context(tc.tile_pool(name="pers", bufs=1))
        ptp = ctx.enter_context(tc.tile_pool(name="ptp", bufs=9))
        hgp = ctx.enter_context(tc.tile_pool(name="hgp", bufs=2))
        stat = ctx.enter_context(tc.tile_pool(name="stat", bufs=4))
        ctxp = ctx.enter_context(tc.tile_pool(name="ctxp", bufs=3))
        psP = ctx.enter_context(tc.tile_pool(name="psP", bufs=2, space="PSUM"))
        psS = ctx.enter_context(tc.tile_pool(name="psS", bufs=2, space="PSUM"))
        psC = ctx.enter_context(tc.tile_pool(name="psC", bufs=2, space="PSUM"))

        # ---- persistent SBUF tiles ----
        cst = pers.tile([128, NCST], F32, name="cst", tag="cst")
        nc.sync.dma_start(out=cst, in_=d_cst)
        co = 0
        def cslice(n):
            nonlocal co
            a = cst[:, co:co + n]; co += n
            return a
        qb = [cslice(1) for _ in range(2)]
        fb1 = [cslice(1) for _ in range(8)]
        nb = [[cslice(1) for _ in range(2)] for _ in range(2)]
        ga1 = cslice(NS)
        gAp = [[cslice(NS) for _ in range(2)] for _ in range(2)]
        gBp = [[cslice(NS) for _ in range(2)] for _ in range(2)]
        km = [cslice(1) for _ in range(NKT)]
        kmi = {}
        idx = 0
        for s in range(NS):
            for kt in range(nkt[s]):
                kmi[(s, kt)] = idx; idx += 1

        # Few LARGE DMAs, critical-first: each DMACopy pays ~1.4us of fixed
        # HWDGE/sem overhead, so slot-granular loads serialize the startup.
        q_off = int(offs[NS // 4])
        half_off = int(offs[NS // 2])
        ident = pers.tile([128, 128], BF16, name="ident", tag="ident")
        nc.sync.dma_start(out=ident, in_=d_id)
        xt = [pers.tile([128, R], BF16, name=f"xt{i}", tag=f"xt{i}") for i in range(2)]
        for ct in range(2):
            nc.sync.dma_start(out=xt[ct][:, 0:q_off], in_=d_xt[ct][:, 0:q_off])
        wqkt = pers.tile([128, 2, 512], BF16, name="wqkt", tag="wqkt")
        nc.sync.dma_start(out=wqkt, in_=d_wqk)
        wvot = pers.tile([128, 2, 264], BF16, name="wvot", tag="wvot")
        nc.sync.dma_start(out=wvot, in_=d_wvo)
        for ct in range(2):
            nc.sync.dma_start(out=xt[ct][:, q_off:half_off], in_=d_xt[ct][:, q_off:half_off])
        for ct in range(2):
            nc.sync.dma_start(out=xt[ct][:, half_off:R], in_=d_xt[ct][:, half_off:R])
        wot = pers.tile([128, 2, 2, 128], FP8, name="wot", tag="wot")
        nc.sync.dma_start(out=wot, in_=d_wo)
        w1 = pers.tile([128, 2, 1024], FP8, name="w1", tag="w1")
        nc.sync.dma_start(out=w1, in_=d_w1)
        w2t = pers.tile([128, 8, 2, 128], FP8, name="w2t", tag="w2t")
        nc.sync.dma_start(out=w2t, in_=d_w2)

        # PE p-state warmup: ~24 dep-free matmuls keep the PE continuously
        # busy from t~1us so real matmuls start at the fast clock.
        for _ in range(36):
            wps = psP.tile([128, 512], F32, name="psp", tag="psp")
            mm(wps[:, :128], ident, ident, start=True, stop=True)

        xn = [pers.tile([128, R], BF16, name=f"xn{i}", tag=f"xn{i}") for i in range(2)]
        qt_ = [pers.tile([128, R], BF16, name=f"q{i}", tag=f"q{i}") for i in range(2)]
        kt_ = [pers.tile([128, R], BF16, name=f"k{i}", tag=f"k{i}") for i in range(2)]
        qZ = pers.tile([128, 8, MMAX], BF16, name="qZ", tag="qZ")
        nc.gpsimd.memset(qZ, 0.0)  # persistent zeros; head h only ever writes rows 32*(h%4)
        vr = pers.tile([128, 264 * NKT], BF16, name="vr", tag="vr")
        ctxt = pers.tile([128, 2, R], FP8, name="ctxt", tag="ctxt")
        x2 = [pers.tile([128, R], BF16, name=f"x2{i}", tag=f"x2{i}") for i in range(2)]
        for ct in range(2):
            nc.gpsimd.memset(x2[ct], 0.0)  # pads must stay 0 for gnorm2 stats
        xn2 = pers.tile([128, 2, R], FP8, name="xn2", tag="xn2")
        out_t = [pers.tile([128, R], BF16, name=f"ot{i}", tag=f"ot{i}") for i in range(2)]

        # ---------- GraphNorm stats+apply ----------
        # rstd = rsqrt(var) via DVE reciprocal seed + Newton (keeps the ACT
        # engine free of Ln/Exp and their 1283ns table loads; eps ~ 1e-5 on a
        # ~1.0 std is far below the fp8/bf16 noise floor, dropped).
        def gnorm_stats(src_f, widx, slots, tg):
            """bn_stats (DVE only) + var/mean prep; returns (var, meanp)."""
            nsl = len(slots)
            c0 = slots[0]
            var = stat.tile([128, 2, nsl], F32, name="var", tag=f"var{tg}")
            meanp = stat.tile([128, 2, nsl], F32, name="meanp", tag=f"meanp{tg}")
            for ct in range(2):
                mv = stat.tile([128, 2, nsl], F32, name="mv", tag=f"mv{tg}{ct}")
                for i, s in enumerate(slots):
                    st6 = stat.tile([128, 6], F32, name="st6", tag="st6")
                    nc.vector.bn_stats(out=st6, in_=src_f(ct)[:, offs[s]:offs[s] + Ms[s]])
                    nc.vector.bn_aggr(out=mv[:, :, i:i + 1], in_=st6)
                mean_r = mv[:, 0:1, :].squeeze(1)
                var_r = mv[:, 1:2, :].squeeze(1)
                m2 = stat.tile([128, nsl], F32, name="m2", tag="m2")
                nc.vector.tensor_mul(m2, mean_r, mean_r)
                v1 = stat.tile([128, nsl], F32, name="v1", tag="v1")
                nc.vector.tensor_mul(v1, var_r, gAp[widx][ct][:, c0:c0 + nsl])
                nc.vector.tensor_mul(var[:, ct, :], m2, gBp[widx][ct][:, c0:c0 + nsl])
                nc.vector.tensor_add(var[:, ct, :], var[:, ct, :], v1)
                nc.vector.tensor_mul(meanp[:, ct, :], mean_r, ga1[:, c0:c0 + nsl])
            return var, meanp

        def gnorm_chain(var, meanp, widx, slots, tg, eng, recip_seed):
            """rsqrt chain -> (y == scale, per-ct shift). Runs on `eng` so the
            ~6us of serial hop latency doesn't head-of-line block DVE."""
            nsl = len(slots)
            vv = var[:, :, :]
            y = stat.tile([128, 2, nsl], F32, name="y", tag=f"y{tg}")
            t = stat.tile([128, 2, nsl], F32, name="t", tag=f"t{tg}")
            if recip_seed:
                nc.vector.reciprocal_approx_fast(out=y, in_=vv)
                eng.tensor_scalar(out=y, in0=y, scalar1=0.5, scalar2=0.5,
                                  op0=OP.mult, op1=OP.add)
                iters = 1
            else:  # linear seed 1.5 - 0.5v (fine for var in [0.5, 1.6])
                eng.tensor_scalar(out=y, in0=vv, scalar1=-0.5, scalar2=1.5,
                                  op0=OP.mult, op1=OP.add)
                iters = 2
            for _ in range(iters):
                eng.tensor_mul(t, vv, y)
                eng.tensor_mul(t, t, y)
                eng.tensor_scalar(out=t, in0=t, scalar1=-0.5, scalar2=1.5,
                                  op0=OP.mult, op1=OP.add)
                eng.tensor_mul(y, y, t)
            sc_sh = []
            for ct in range(2):
                shift = stat.tile([128, nsl], F32, name="shift", tag=f"shift{tg}{ct}")
                eng.tensor_mul(shift, meanp[:, ct, :], y[:, ct, :])
                eng.tensor_scalar(
                    out=shift, in0=shift, scalar1=-1.0, scalar2=nb[widx][ct],
                    op0=OP.mult, op1=OP.add)
                sc_sh.append(shift)
            return y, sc_sh

        def gnorm_apply(src_f, dst_f, y, sc_sh, slots, awidths, eng, sel=None):
            for i, s in enumerate(slots):
                if sel is not None and s not in sel:
                    continue
                for ct in range(2):
                    w = awidths[s]
                    if eng is nc.scalar:  # ACT: out = Identity(scale*in + bias)
                        nc.scalar.activation(
                            out=dst_f(ct, s, w),
                            in_=src_f(ct)[:, offs[s]:offs[s] + w],
                            func=AF.Identity,
                            bias=sc_sh[ct][:, i:i + 1],
                            scale=y[:, ct, i:i + 1])
                    else:
                        eng.tensor_scalar(
                            out=dst_f(ct, s, w),
                            in0=src_f(ct)[:, offs[s]:offs[s] + w],
                            scalar1=y[:, ct, i:i + 1],
                            scalar2=sc_sh[ct][:, i:i + 1],
                            op0=OP.mult, op1=OP.add)

        def gnorm(src_f, dst_f, widx, slots, awidths, tg,
                  chain_eng=None, apply_eng=None, recip_seed=True):
            var, meanp = gnorm_stats(src_f, widx, slots, tg)
            y, sc_sh = gnorm_chain(var, meanp, widx, slots, tg,
                                   chain_eng or nc.vector, recip_seed)
            gnorm_apply(src_f, dst_f, y, sc_sh, slots, awidths,
                        apply_eng or (nc.vector if widx == 0 else nc.gpsimd))
            return y, sc_sh



        # ---------- phase 2: q,k projections (slot-aligned chunks so each
        # slot's qZ depends only on its own gnorm pair; pads never projected) --
        def qkproj_chunk(o, w, on_act=False):
            for mt in range(4):
                ps = psP.tile([128, 512], F32, name="psp", tag="psp")
                for ktc in range(2):
                    mm(ps[:, :w], wqkt[:, ktc, 128 * mt:128 * mt + 128],
                       xn[ktc][:, o:o + w], start=(ktc == 0), stop=(ktc == 1))
                if mt < 2:  # q: add bias (k bias cancels in softmax)
                    if on_act:
                        nc.scalar.activation(out=qt_[mt][:, o:o + w], in_=ps[:, :w],
                                             func=AF.Identity, bias=qb[mt])
                    else:
                        nc.vector.tensor_scalar_add(qt_[mt][:, o:o + w], ps[:, :w], qb[mt])
                else:
                    if on_act:
                        nc.scalar.activation(out=kt_[mt - 2][:, o:o + w], in_=ps[:, :w],
                                             func=AF.Copy)
                    else:
                        nc.vector.tensor_copy(kt_[mt - 2][:, o:o + w], ps[:, :w])

        def qkproj_slot(s, on_act=False):
            qkproj_chunk(int(offs[s]), int(Ms[s]), on_act)

        vri = {}
        idx = 0
        for s in range(NS):
            for kt in range(nkt[s]):
                vri[(s, kt)] = idx; idx += 1
        def vproj(s, kt, on_act=False):
            mkt = min(128, Ms[s] - 128 * kt)
            ko = offs[s] + 128 * kt
            vb = 264 * vri[(s, kt)]
            ps = psP.tile([128, 512], F32, name="psp", tag="psp")
            for ctc in range(2):
                mm(ps[:mkt, :264], xn[ctc][:, ko:ko + mkt], wvot[:, ctc],
                   start=(ctc == 0), stop=(ctc == 1))
            if on_act:
                nc.scalar.activation(out=vr[:mkt, vb:vb + 264], in_=ps[:mkt, :264],
                                     func=AF.Copy)
            else:
                nc.vector.tensor_copy(vr[:mkt, vb:vb + 264], ps[:mkt, :264])
            # sumexp ones-column = 1/SCX (ctx fp8 prescale rides the ratio)
            ones = vr[:mkt, vb:vb + 264].rearrange("p (h c) -> p h c", h=8, c=33)[:, :, 32:33].squeeze(2)
            nc.gpsimd.memset(ones, 1.0 / SCX)

        # ---------- attention, software-pipelined over (slot, qtile) ----------
        def qz_slot(s, on_dve):
            M = Ms[s]
            eng = nc.vector if on_dve else nc.gpsimd
            for h in range(8):
                hp = 32 * (h % 4)
                eng.tensor_copy(qZ[hp:hp + 32, h, :M],
                                qt_[h // 4][hp:hp + 32, offs[s]:offs[s] + M])

        def attn_front(s, qi):
            """scores + exp for all key tiles of (s, qi); returns state."""
            M = Ms[s]; nv = ns_valid[s]
            qo = 128 * qi
            qc = min(128, nv - qo)
            pts = []
            for kt in range(nkt[s]):
                mkt = min(128, M - 128 * kt)
                ko = offs[s] + 128 * kt
                st = psS.tile([128, 1024], F32, name="st", tag="st")
                for h in range(8):
                    mm(st[:mkt, 128 * h:128 * h + qc],
                       kt_[h // 4][:, ko:ko + mkt], qZ[:, h, qo:qo + qc],
                       start=True, stop=True)
                pt = ptp.tile([128, 1024], BF16, name="pt", tag="pt")
                stv = st[:mkt, :].rearrange("p (h c) -> p h c", h=8, c=128)[:, :, :qc]
                ptv = pt[:mkt, :].rearrange("p (h c) -> p h c", h=8, c=128)[:, :, :qc]
                nc.scalar.activation(out=ptv, in_=stv, func=AF.Exp,
                                     bias=km[kmi[(s, kt)]][:mkt], scale=SC)
                pts.append(pt)
            return pts

        def attn_back(s, qi, pts):
            """PV + divide + transpose + ctxt store for (s, qi)."""
            M = Ms[s]; nv = ns_valid[s]
            qo = 128 * qi
            qc = min(128, nv - qo)
            qbase = offs[s] + qo
            # cs ([128,264] f32) + tp ([128,2,128] bf16 via bitcast) share
            # one PSUM bank: 1056B + 512B < 2KB
            csbank = psC.tile([128, 392], F32, name="csbank", tag="cs")
            cs = csbank[:, 0:264]
            tp = csbank[:, 264:392].bitcast(BF16).rearrange(
                "p (t c) -> p t c", t=2, c=128)
            for kt in range(nkt[s]):
                mkt = min(128, M - 128 * kt)
                vb = 264 * vri[(s, kt)]
                last = kt == nkt[s] - 1
                for h in range(8):
                    mm(cs[:qc, 33 * h:33 * h + 33],
                       pts[kt][:mkt, 128 * h:128 * h + qc],
                       vr[:mkt, vb + 33 * h:vb + 33 * h + 33],
                       start=(kt == 0), stop=last)
            # rec = SCX / sumexp ; ctxT = cs * rec (broadcast over 33-blocks)
            rec = stat.tile([128, 8], F32, name="rec", tag="rec")
            den = cs[:qc, :].rearrange("p (h c) -> p h c", h=8, c=33)[:, :, 32:33].squeeze(2)
            nc.vector.reciprocal_approx_fast(out=rec[:qc, :], in_=den)
            ctxTs = ctxp.tile([128, 256], BF16, name="ctxTs", tag="ctxTs")
            csv = cs[:qc, :].rearrange("p (h c) -> p h c", h=8, c=33)[:, :, 0:32]
            ctv = ctxTs[:qc, :].rearrange("p (h c) -> p h c", h=8, c=32)
            rv = rec[:qc, :].unsqueeze(2).broadcast_to([qc, 8, 32])
            nc.vector.tensor_tensor(out=ctv, in0=csv, in1=rv, op=OP.mult)
            for ctc in range(2):
                nc.tensor.transpose(tp[:, ctc, :qc], ctxTs[:qc, 128 * ctc:128 * ctc + 128],
                                    ident[:qc, :qc])
            nc.vector.tensor_copy(ctxt[:, :, qbase:qbase + qc], tp[:, :, :qc])

        # ---------- out_proj + residual (valid width only; pads stay 0) ----
        def outproj_slot(s):
            nv = ns_valid[s]
            o = offs[s]
            for ctc in range(2):
                ps = psP.tile([128, 512], F32, name="psp", tag="psp")
                for cw in range(0, nv, 256):  # DoubleRow rhs free = 2*w <= 512
                    w = min(256, nv - cw)
                    mm(ps[:, cw:cw + w], wot[:, ctc], ctxt[:, :, o + cw:o + cw + w],
                       start=True, stop=True, perf_mode=PM.DoubleRow)
                nc.vector.scalar_tensor_tensor(
                    out=x2[ctc][:, o:o + nv], in0=ps[:, :nv], scalar=1.0 / (SCX * SO),
                    in1=xt[ctc][:, o:o + nv], op0=OP.mult, op1=OP.add)

        # ---------- FFN, two stages for slot-level staggering ----------
        def ffn1_slot(s, pair_gelu):
            nv = ns_valid[s]
            o = offs[s]
            hg = [hgp.tile([128, 2, MMAX], FP8, name=f"hg{p}", tag=f"hg{p}") for p in range(4)]
            for p in range(4):
                if pair_gelu:
                    # one [128,1024] psum holds the mt-pair; one Gelu covers both
                    ps = psS.tile([128, 1024], F32, name="st", tag="st")
                    for half in range(2):
                        mt = 2 * p + half
                        for cw in range(0, nv, 256):
                            w = min(256, nv - cw)
                            mm(ps[:, 512 * half + cw:512 * half + cw + w],
                               w1[:, :, 128 * mt:128 * mt + 128],
                               xn2[:, :, o + cw:o + cw + w],
                               start=True, stop=True, perf_mode=PM.DoubleRow)
                    psv = ps[:, :].rearrange("p (t c) -> p t c", t=2, c=512)[:, :, :nv]
                    nc.scalar.activation(out=hg[p][:, :, :nv], in_=psv,
                                         func=AF.Gelu, scale=1.0 / S1)
                else:
                    for half in range(2):
                        mt = 2 * p + half
                        ps = psP.tile([128, 512], F32, name="psp", tag="psp")
                        for cw in range(0, nv, 256):
                            w = min(256, nv - cw)
                            mm(ps[:, cw:cw + w], w1[:, :, 128 * mt:128 * mt + 128],
                               xn2[:, :, o + cw:o + cw + w],
                               start=True, stop=True, perf_mode=PM.DoubleRow)
                        nc.scalar.activation(out=hg[p][:, half, :nv], in_=ps[:, :nv],
                                             func=AF.Gelu, bias=fb1[mt], scale=1.0 / S1)
            return hg

        def ffn2_slot(s, hg):
            nv = ns_valid[s]
            o = offs[s]
            for ctc in range(2):
                ps2 = psP.tile([128, 512], F32, name="psp", tag="psp")
                for cw in range(0, nv, 256):
                    w = min(256, nv - cw)
                    for p in range(4):
                        mm(ps2[:, cw:cw + w], w2t[:, 4 * ctc + p], hg[p][:, :, cw:cw + w],
                           start=(p == 0), stop=(p == 3), perf_mode=PM.DoubleRow)
                nc.vector.scalar_tensor_tensor(
                    out=out_t[ctc][:, o:o + nv], in0=ps2[:, :nv], scalar=1.0 / S2,
                    in1=x2[ctc][:, o:o + nv], op0=OP.mult, op1=OP.add)

        # ---------- main pipelined emission ----------
        # attention staggered by one (s, qi) item: scores/exp of item i+1 sit
        # ahead of PV(i) in the PE queue, so PE never head-of-line blocks on
        # an exp. FFN staggered by one slot for the same reason. ACT stream
        # stays [exps..., gelus...] to avoid 1283ns table reloads.
        # startup: absolute minimum before the first scores/exp of slot 0 —
        # gnorm1 pair {s0,s1}, projections of s0/s1, qZ(0), vproj(0). The
        # other pairs are emitted inside the first loop iterations and
        # pipeline their (stats -> rsqrt chain -> apply -> proj) latency
        # behind the running exp stream.
        xn_dst = lambda ct, s, w: xn[ct][:, offs[s]:offs[s] + w]
        xt_src = lambda ct: xt[ct]
        gnorm(xt_src, xn_dst, 0, [0, 1], {s: Ms[s] for s in range(NS)}, "g1p0")
        qkproj_slot(0, on_act=True)
        qz_slot(0, True)
        qkproj_slot(1, on_act=True)
        for kt in range(nkt[0]):
            vproj(0, kt, on_act=True)
        qz_slot(1, True)
        for kt in range(nkt[1]):
            vproj(1, kt, on_act=True)
        def deferred_startup(i):
            if i in (0, 1, 2):
                p = i + 1
                gnorm(xt_src, xn_dst, 0, [2 * p, 2 * p + 1],
                      {s: Ms[s] for s in range(NS)}, f"g1p{p}")
                qkproj_slot(2 * p)
                qkproj_slot(2 * p + 1)

        items = [(s, qi) for s in range(NS) for qi in range(math.ceil(ns_valid[s] / 128))]
        DEPTH = 2
        fronts = {}
        g2_half = {}

        def do_back(j):
            sj, qj = items[j]
            attn_back(sj, qj, fronts.pop(j))
            if qj == math.ceil(ns_valid[sj] / 128) - 1:
                outproj_slot(sj)
                if sj == 3:
                    g2_half[0] = gnorm(
                        lambda ct: x2[ct],
                        lambda ct, s2, w: xn2[:, ct, offs[s2]:offs[s2] + w],
                        1, [0, 1, 2, 3], {t: ns_valid[t] for t in range(NS)}, "g2a",
                        chain_eng=nc.gpsimd, recip_seed=False)

        for i, (s, qi) in enumerate(items):
            deferred_startup(i)
            if qi == 0 and s + 2 < NS:
                qz_slot(s + 2, False)
                for kt in range(nkt[s + 2]):
                    vproj(s + 2, kt)
            fronts[i] = attn_front(s, qi)
            if i >= DEPTH:
                do_back(i - DEPTH)
        for j in range(len(items) - DEPTH, len(items)):
            do_back(j)
        gnorm(lambda ct: x2[ct],
              lambda ct, s2, w: xn2[:, ct, offs[s2]:offs[s2] + w],
              1, [4, 5, 6, 7], {t: ns_valid[t] for t in range(NS)}, "g2b",
              chain_eng=nc.gpsimd, recip_seed=False)
        hgprev = None
        for s in range(NS):
            hgnew = ffn1_slot(s, pair_gelu)
            if hgprev is not None:
                ffn2_slot(s - 1, hgprev)
            hgprev = hgnew
            if s == NS // 2:
                for ctc in range(2):
                    nc.sync.dma_start(out=d_ot[ctc][:, 0:offs[NS // 2]],
                                      in_=out_t[ctc][:, 0:offs[NS // 2]])
        ffn2_slot(NS - 1, hgprev)
        for ctc in range(2):
            nc.sync.dma_start(out=d_ot[ctc][:, offs[NS // 2]:R],
                              in_=out_t[ctc][:, offs[NS // 2]:R])
    return nc


_CACHE = {}


def _prepare(inputs):
    x = np.asarray(inputs["x"], np.float32)
    batch = np.asarray(inputs["batch"]).astype(np.int64)
    counts, starts, slot_graph, Ms, offs, Rtot, R = _plan(batch)
    NS = len(Ms)
    nkt = [math.ceil(m / 128) for m in Ms]
    NKT = sum(nkt)

    in_proj_w = np.asarray(inputs["in_proj_w"], np.float32)
    in_proj_b = np.asarray(inputs["in_proj_b"], np.float32)
    out_proj_w = np.asarray(inputs["out_proj_w"], np.float32)
    out_proj_b = np.asarray(inputs["out_proj_b"], np.float32)
    ffn_w1 = np.asarray(inputs["ffn_w1"], np.float32)
    ffn_b1 = np.asarray(inputs["ffn_b1"], np.float32)
    ffn_w2 = np.asarray(inputs["ffn_w2"], np.float32)
    ffn_b2 = np.asarray(inputs["ffn_b2"], np.float32)

    # biases folded into the residual stream (gnorm is shift-invariant):
    # x' = x + out_proj_b + wo @ v_bias + ffn_b2
    fold = out_proj_b + out_proj_w @ in_proj_b[2 * H:3 * H] + ffn_b2
    xb = x + fold[None, :]

    wqk = np.ascontiguousarray(
        in_proj_w[:2 * H].T.reshape(2, 128, 512).transpose(1, 0, 2)).astype(ml_dtypes.bfloat16)
    # wv expanded to 33-col stride with zero ones-columns
    wvT = in_proj_w[2 * H:].T.reshape(2, 128, 8, 32)
    wvo = np.zeros((2, 128, 8, 33), np.float32)
    wvo[:, :, :, :32] = wvT
    wvo = np.ascontiguousarray(
        wvo.reshape(2, 128, 264).transpose(1, 0, 2)).astype(ml_dtypes.bfloat16)
    # wo fp8 [ct_out][128, 2(plane=ct_in), 128], prescaled
    woT = (out_proj_w.T * SO).reshape(2, 128, 2, 128)   # [ct_in, part, ct_out, col]
    wo8 = np.ascontiguousarray(woT.transpose(1, 2, 0, 3)).astype(ml_dtypes.float8_e4m3)
    # w1 fp8 [128, 2, 1024]: plane = input ct
    w18 = np.ascontiguousarray((ffn_w1.T * S1).reshape(2, 128, 1024).transpose(1, 0, 2)).astype(ml_dtypes.float8_e4m3)
    # w2 fp8 [8][128, 2, 128]: idx = 4*ct_out + pair; plane i = hidden 256p+128i
    w2T = (ffn_w2.T * S2).reshape(4, 2, 128, 2, 128)    # [pair, plane, part, ct_out, col]
    # [part, idx=4*ct_out+pair, plane, col]
    w28 = np.ascontiguousarray(
        w2T.transpose(2, 3, 0, 1, 4).reshape(128, 8, 2, 128)).astype(ml_dtypes.float8_e4m3)

    qkb = in_proj_b[:2 * H].reshape(4, 128)
    fb1 = ffn_b1.reshape(8, 128)
    nwv = np.stack([np.asarray(inputs["norm1_w"], np.float32).reshape(2, 128),
                    np.asarray(inputs["norm2_w"], np.float32).reshape(2, 128)])
    nbv = np.stack([np.asarray(inputs["norm1_b"], np.float32).reshape(2, 128),
                    np.asarray(inputs["norm2_b"], np.float32).reshape(2, 128)])

    xT = xb.T  # [256, N]
    xts = np.zeros((N_CORES, 2, 128, R), np.float32)
    ga1 = np.zeros((N_CORES, 128, NS), np.float32)
    gA = np.zeros((N_CORES, 128, NS), np.float32)
    gB = np.zeros((N_CORES, 128, NS), np.float32)
    kms = np.full((N_CORES, NKT, 128), NEG, np.float32)
    ns_valid = [0] * NS
    for c in range(N_CORES):
        for s in range(NS):
            g = slot_graph[c, s]
            n = int(counts[g])
            st = int(starts[g])
            o = int(offs[s])
            ns_valid[s] = max(ns_valid[s], n)
            if n > 0:
                blk = xT[:, st:st + n]
                xts[c, 0, :, o:o + n] = blk[:128]
                xts[c, 1, :, o:o + n] = blk[128:]
            ne = max(n, 1)
            ga1[c, :, s] = Ms[s] / ne
            inv_nm1 = 1.0 / max(ne - 1, 1)
            gA[c, :, s] = Ms[s] * inv_nm1
            gB[c, :, s] = Ms[s] * (1.0 - Ms[s] / ne) * inv_nm1
            ki = sum(nkt[:s])
            for kt in range(nkt[s]):
                v = min(128, max(0, n - 128 * kt))
                kms[c, ki + kt, :v] = 0.0
    ns_valid = [int(math.ceil(v / 16) * 16) if v % 16 else v for v in ns_valid]
    ns_valid = [min(v, Ms[s]) for s, v in enumerate(ns_valid)]

    pair_gelu = bool(np.all(ffn_b1 == 0))
    key = (tuple(Ms), R, tuple(ns_valid), pair_gelu)
    if key not in _CACHE:
        nc = bacc.Bacc("TRN2", target_bir_lowering=False, debug=False,
                       num_devices=N_CORES)
        _build(nc, Ms, offs, R, ns_valid, pair_gelu)
        nc.compile()
        _CACHE[key] = nc
    nc = _CACHE[key]

    ident = np.eye(128).astype(ml_dtypes.bfloat16)
    in_maps = []
    for c in range(N_CORES):
        # norm_w folded: y = rsqrt(var / nw^2) = nw * rsqrt(var) (nw >= 0)
        nw2 = np.maximum(nwv.reshape(2, 2, 128), 1e-20) ** 2  # [phase, ct, 128]
        gparts = []
        for gsrc in (gA[c], gB[c]):           # [128, NS]
            for ph in range(2):
                for ctc in range(2):
                    gparts.append(gsrc / nw2[ph, ctc][:, None])
        cstc = np.concatenate(
            [qkb[:2].T,                       # qb (q bias only)
             fb1.T,                           # 8
             nbv.reshape(4, 128).T,           # 4
             ga1[c]] + gparts +               # NS + 8*NS
            [kms[c].T], axis=1).astype(np.float32)
        in_maps.append({
            "xt": xts[c].astype(ml_dtypes.bfloat16),
            "wqk": wqk, "wvo": wvo, "wo": wo8, "w1": w18, "w2": w28,
            "cst": np.ascontiguousarray(cstc),
            "ident": ident,
        })

    def unpack(outs):
        out = np.empty((x.shape[0], H), np.float32)
        for c in range(N_CORES):
            ot = np.asarray(outs[c]["ot"]).astype(np.float32)  # [2, 128, R]
            full = np.concatenate([ot[0], ot[1]], axis=0)      # [256, R]
            for s in range(NS):
                g = slot_graph[c, s]
                n = int(counts[g])
                st = int(starts[g])
                o = int(offs[s])
                if n > 0:
                    out[st:st + n] = full[:, o:o + n].T
        return out

    return nc, in_maps, unpack


def kernel(**inputs):
    nc, in_maps, unpack = _prepare(inputs)
    res = run_bass_kernel_spmd(nc, in_maps, list(range(N_CORES)))
    return unpack(res.results)


def _traced_run(**inputs):
    """Cost-model timeline (single core) + warm wall-clock. Returns model ns."""
    import time
    nc, in_maps, unpack = _prepare(inputs)
    t0 = time.time()
    run_bass_kernel_spmd(nc, in_maps, list(range(N_CORES)))
    t1 = time.time()
    run_bass_kernel_spmd(nc, in_maps, list(range(N_CORES)))
    t2 = time.time()
    print(f"wall cold: {t1 - t0:.2f}s  warm: {t2 - t1:.2f}s")
    from concourse.timeline_sim import TimelineSim
    import trails.perfetto as _tp
    for _m in ("enable_explicit_ordering", "reserve_process_order",
               "reserve_thread_order", "set_process_order", "set_thread_order",
               "add_instant"):
        if not hasattr(_tp.LazyPerfetto, _m):
            setattr(_tp.LazyPerfetto, _m, lambda self, *a, **k: None)
    if not hasattr(_tp.LazyPerfetto, "add_counter"):
        def _add_counter(self, *a, **k):
            try:
                self.update_counter(*a, **k)
            except Exception:
                pass
        _tp.LazyPerfetto.add_counter = _add_counter
    tl = TimelineSim(nc, trace=True)
    total = tl.simulate()
    pf = tl.perfetto
    if callable(pf):
        pf = pf()
    if pf is not None:
        try:
            pf.save("/root/problem/tl.perfetto-trace")
        except Exception as e:
            print("perfetto dump failed:", e)
    return total


# revision 44
# speedup vs baseline: 1.1146x; 1.0001x over previous
"""Trainium2 Bass kernel for EnhancedTransformerBlock on ragged graphs.

Layout: transposed activations [channels (partitions), nodes (free)].
Sharding: 64 graphs -> 8 cores x 8 slots, assigned by size-sorted rank so
slot widths (uniform across cores, required for SPMD) hug the max count.

v2 design notes (vs the phase-batched f32r baseline):
- all matmul activations bf16; FFN + out_proj in fp8e4m3 with DoubleRow
  (contraction 256 per matmul at 0.5 cyc/row).
- scores: zero-padded per-head q replica (qZ) built once per slot with 4x-mode
  DVE copies; contraction 128 (4 heads of k x zero-trick).
- PV transposed: out [q<=128, 33] per head; col 33h+32 of vr holds 0.125 so the
  same matmul accumulates sumexp/8 (fp8 range prep for ctx).
- divide = stride-0 broadcast tensor_tensor; back-transpose on PE (identity).
- k needs no bias (cancels in softmax over keys); out_proj bias + wo@v_bias +
  ffn_b2 pre-added to x on host (GraphNorm is per-channel shift invariant).
- per-slot pipelining: attention(s) -> out_proj(s) -> gnorm2 stats(s); FFN per
  half interleaved with the other half's attention.
"""

import math
import numpy as np
import ml_dtypes

import concourse.bass as bass
import concourse.bacc as bacc
import concourse.mybir as mybir
import concourse.tile as tile
from concourse.bass_utils import run_bass_kernel_spmd
from contextlib import ExitStack

N_CORES = 8
B = 64
H = 256
NH = 8
HD = H // NH
EPS = 1e-5

F32 = mybir.dt.float32
BF16 = mybir.dt.bfloat16
FP8 = mybir.dt.float8e4
AF = mybir.ActivationFunctionType
OP = mybir.AluOpType
PM = mybir.MatmulPerfMode

NEG = -30.0          # additive key mask (pre-exp); exp(-30) == 0 in bf16
SC = 1.0 / math.sqrt(HD)
S1 = 32.0            # ffn_w1 fp8 prescale
S2 = 32.0            # ffn_w2 fp8 prescale
SO = 32.0            # out_proj_w fp8 prescale
SCX = 8.0            # ctx fp8 prescale (via 1/8 in the vr ones-column)


def _plan(batch):
    batch = np.asarray(batch).astype(np.int64)
    counts = np.bincount(batch, minlength=B).astype(np.int64)
    starts = np.concatenate([[0], np.cumsum(counts)[:-1]])
    order = np.argsort(-counts, kind="stable")  # rank -> graph id
    NS = B // N_CORES
    Ms, slot_graph = [], np.zeros((N_CORES, NS), np.int64)
    for s in range(NS):
        blk = order[N_CORES * s: N_CORES * s + N_CORES]
        m = int(max(16, math.ceil(max(1, counts[blk].max()) / 16) * 16))
        Ms.append(m)
        for c in range(N_CORES):
            slot_graph[c, s] = blk[c]
    offs = np.concatenate([[0], np.cumsum(Ms)]).astype(np.int64)
    Rtot = int(offs[-1])
    R = int(math.ceil(Rtot / 128) * 128)
    return counts, starts, slot_graph, Ms, offs, Rtot, R


def _build(nc, Ms, offs, R, ns_valid, pair_gelu):
    """ns_valid[s] = max valid node count over cores for slot s (<= Ms[s]).
    Per-core valid counts differ; we compute the slot at the max width and the
    km mask (per core) zeroes the prob rows beyond each core's own count.
    Query-side trims use ns_valid (same extent every core keeps SPMD single
    program); pads beyond ns_valid are never read back by any core."""
    NS = len(Ms)
    nkt = [math.ceil(m / 128) for m in Ms]
    NKT = sum(nkt)
    MMAX = max(Ms)

    # ---- DRAM ----
    d_xt = nc.dram_tensor("xt", [2, 128, R], BF16, kind="ExternalInput").ap()
    d_wqk = nc.dram_tensor("wqk", [128, 2, 512], BF16, kind="ExternalInput").ap()
    d_wvo = nc.dram_tensor("wvo", [128, 2, 264], BF16, kind="ExternalInput").ap()
    d_wo = nc.dram_tensor("wo", [128, 2, 2, 128], FP8, kind="ExternalInput").ap()
    d_w1 = nc.dram_tensor("w1", [128, 2, 1024], FP8, kind="ExternalInput").ap()
    d_w2 = nc.dram_tensor("w2", [128, 8, 2, 128], FP8, kind="ExternalInput").ap()
    # packed per-partition constants:
    # [qb(2) fb1(8) nb(4) ga1(NS) gAp(4*NS) gBp(4*NS) km(NKT)]
    NCST = 14 + 9 * NS + NKT
    d_cst = nc.dram_tensor("cst", [128, NCST], F32, kind="ExternalInput").ap()
    d_id = nc.dram_tensor("ident", [128, 128], BF16, kind="ExternalInput").ap()
    d_ot = nc.dram_tensor("ot", [2, 128, R], BF16, kind="ExternalOutput").ap()

    mm = nc.tensor.matmul

    with tile.TileContext(nc) as tc, ExitStack() as ctx:
        pers = ctx.enter_context(tc.tile_pool(name="pers", bufs=1))
        ptp = ctx.enter_context(tc.tile_pool(name="ptp", bufs=12))
        hgp = ctx.enter_context(tc.tile_pool(name="hgp", bufs=2))
        stat = ctx.enter_context(tc.tile_pool(name="stat", bufs=4))
        ctxp = ctx.enter_context(tc.tile_pool(name="ctxp", bufs=3))
        psP = ctx.enter_context(tc.tile_pool(name="psP", bufs=2, space="PSUM"))
        psS = ctx.enter_context(tc.tile_pool(name="psS", bufs=2, space="PSUM"))
        psC = ctx.enter_context(tc.tile_pool(name="psC", bufs=2, space="PSUM"))

        # ---- persistent SBUF tiles ----
        cst = pers.tile([128, NCST], F32, name="cst", tag="cst")
        nc.sync.dma_start(out=cst, in_=d_cst)
        co = 0
        def cslice(n):
            nonlocal co
            a = cst[:, co:co + n]; co += n
            return a
        qb = [cslice(1) for _ in range(2)]
        fb1 = [cslice(1) for _ in range(8)]
        nb = [[cslice(1) for _ in range(2)] for _ in range(2)]
        ga1 = cslice(NS)
        gAp = [[cslice(NS) for _ in range(2)] for _ in range(2)]
        gBp = [[cslice(NS) for _ in range(2)] for _ in range(2)]
        km = [cslice(1) for _ in range(NKT)]
        kmi = {}
        idx = 0
        for s in range(NS):
            for kt in range(nkt[s]):
                kmi[(s, kt)] = idx; idx += 1

        # Few LARGE DMAs, critical-first: each DMACopy pays ~1.4us of fixed
        # HWDGE/sem overhead, so slot-granular loads serialize the startup.
        q_off = int(offs[NS // 4])
        half_off = int(offs[NS // 2])
        ident = pers.tile([128, 128], BF16, name="ident", tag="ident")
        nc.sync.dma_start(out=ident, in_=d_id)
        xt = [pers.tile([128, R], BF16, name=f"xt{i}", tag=f"xt{i}") for i in range(2)]
        for ct in range(2):
            nc.sync.dma_start(out=xt[ct][:, 0:q_off], in_=d_xt[ct][:, 0:q_off])
        wqkt = pers.tile([128, 2, 512], BF16, name="wqkt", tag="wqkt")
        nc.sync.dma_start(out=wqkt, in_=d_wqk)
        wvot = pers.tile([128, 2, 264], BF16, name="wvot", tag="wvot")
        nc.sync.dma_start(out=wvot, in_=d_wvo)
        for ct in range(2):
            nc.sync.dma_start(out=xt[ct][:, q_off:half_off], in_=d_xt[ct][:, q_off:half_off])
        for ct in range(2):
            nc.sync.dma_start(out=xt[ct][:, half_off:R], in_=d_xt[ct][:, half_off:R])
        wot = pers.tile([128, 2, 2, 128], FP8, name="wot", tag="wot")
        nc.sync.dma_start(out=wot, in_=d_wo)
        w1 = pers.tile([128, 2, 1024], FP8, name="w1", tag="w1")
        nc.sync.dma_start(out=w1, in_=d_w1)
        w2t = pers.tile([128, 8, 2, 128], FP8, name="w2t", tag="w2t")
        nc.sync.dma_start(out=w2t, in_=d_w2)

        # PE p-state warmup: ~24 dep-free matmuls keep the PE continuously
        # busy from t~1us so real matmuls start at the fast clock.
        for _ in range(36):
            wps = psP.tile([128, 512], F32, name="psp", tag="psp")
            mm(wps[:, :128], ident, ident, start=True, stop=True)

        xn = [pers.tile([128, R], BF16, name=f"xn{i}", tag=f"xn{i}") for i in range(2)]
        qt_ = [pers.tile([128, R], BF16, name=f"q{i}", tag=f"q{i}") for i in range(2)]
        kt_ = [pers.tile([128, R], BF16, name=f"k{i}", tag=f"k{i}") for i in range(2)]
        qZ = pers.tile([128, 8, MMAX], BF16, name="qZ", tag="qZ")
        nc.gpsimd.memset(qZ, 0.0)  # persistent zeros; head h only ever writes rows 32*(h%4)
        vr = pers.tile([128, 264 * NKT], BF16, name="vr", tag="vr")
        ctxt = pers.tile([128, 2, R], FP8, name="ctxt", tag="ctxt")
        x2 = [pers.tile([128, R], BF16, name=f"x2{i}", tag=f"x2{i}") for i in range(2)]
        for ct in range(2):
            nc.gpsimd.memset(x2[ct], 0.0)  # pads must stay 0 for gnorm2 stats
        xn2 = pers.tile([128, 2, R], FP8, name="xn2", tag="xn2")
        out_t = [pers.tile([128, R], BF16, name=f"ot{i}", tag=f"ot{i}") for i in range(2)]

        # ---------- GraphNorm stats+apply ----------
        # rstd = rsqrt(var) via DVE reciprocal seed + Newton (keeps the ACT
        # engine free of Ln/Exp and their 1283ns table loads; eps ~ 1e-5 on a
        # ~1.0 std is far below the fp8/bf16 noise floor, dropped).
        def gnorm_stats(src_f, widx, slots, tg):
            """bn_stats (DVE only) + var/mean prep; returns (var, meanp)."""
            nsl = len(slots)
            c0 = slots[0]
            var = stat.tile([128, 2, nsl], F32, name="var", tag=f"var{tg}")
            meanp = stat.tile([128, 2, nsl], F32, name="meanp", tag=f"meanp{tg}")
            for ct in range(2):
                mv = stat.tile([128, 2, nsl], F32, name="mv", tag=f"mv{tg}{ct}")
                for i, s in enumerate(slots):
                    st6 = stat.tile([128, 6], F32, name="st6", tag="st6")
                    nc.vector.bn_stats(out=st6, in_=src_f(ct)[:, offs[s]:offs[s] + Ms[s]])
                    nc.vector.bn_aggr(out=mv[:, :, i:i + 1], in_=st6)
                mean_r = mv[:, 0:1, :].squeeze(1)
                var_r = mv[:, 1:2, :].squeeze(1)
                m2 = stat.tile([128, nsl], F32, name="m2", tag="m2")
                nc.vector.tensor_mul(m2, mean_r, mean_r)
                v1 = stat.tile([128, nsl], F32, name="v1", tag="v1")
                nc.vector.tensor_mul(v1, var_r, gAp[widx][ct][:, c0:c0 + nsl])
                nc.vector.tensor_mul(var[:, ct, :], m2, gBp[widx][ct][:, c0:c0 + nsl])
                nc.vector.tensor_add(var[:, ct, :], var[:, ct, :], v1)
                nc.vector.tensor_mul(meanp[:, ct, :], mean_r, ga1[:, c0:c0 + nsl])
            return var, meanp

        def gnorm_chain(var, meanp, widx, slots, tg, eng, recip_seed):
            """rsqrt chain -> (y == scale, per-ct shift). Runs on `eng` so the
            ~6us of serial hop latency doesn't head-of-line block DVE."""
            nsl = len(slots)
            vv = var[:, :, :]
            y = stat.tile([128, 2, nsl], F32, name="y", tag=f"y{tg}")
            t = stat.tile([128, 2, nsl], F32, name="t", tag=f"t{tg}")
            if recip_seed:
                nc.vector.reciprocal_approx_fast(out=y, in_=vv)
                eng.tensor_scalar(out=y, in0=y, scalar1=0.5, scalar2=0.5,
                                  op0=OP.mult, op1=OP.add)
                iters = 1
            else:  # linear seed 1.5 - 0.5v (fine for var in [0.5, 1.6])
                eng.tensor_scalar(out=y, in0=vv, scalar1=-0.5, scalar2=1.5,
                                  op0=OP.mult, op1=OP.add)
                iters = 2
            for _ in range(iters):
                eng.tensor_mul(t, vv, y)
                eng.tensor_mul(t, t, y)
                eng.tensor_scalar(out=t, in0=t, scalar1=-0.5, scalar2=1.5,
                                  op0=OP.mult, op1=OP.add)
                eng.tensor_mul(y, y, t)
            sc_sh = []
            for ct in range(2):
                shift = stat.tile([128, nsl], F32, name="shift", tag=f"shift{tg}{ct}")
                eng.tensor_mul(shift, meanp[:, ct, :], y[:, ct, :])
                eng.tensor_scalar(
                    out=shift, in0=shift, scalar1=-1.0, scalar2=nb[widx][ct],
                    op0=OP.mult, op1=OP.add)
                sc_sh.append(shift)
            return y, sc_sh

        def gnorm_apply(src_f, dst_f, y, sc_sh, slots, awidths, eng, sel=None):
            for i, s in enumerate(slots):
                if sel is not None and s not in sel:
                    continue
                for ct in range(2):
                    w = awidths[s]
                    if eng is nc.scalar:  # ACT: out = Identity(scale*in + bias)
                        nc.scalar.activation(
                            out=dst_f(ct, s, w),
                            in_=src_f(ct)[:, offs[s]:offs[s] + w],
                            func=AF.Identity,
                            bias=sc_sh[ct][:, i:i + 1],
                            scale=y[:, ct, i:i + 1])
                    else:
                        eng.tensor_scalar(
                            out=dst_f(ct, s, w),
                            in0=src_f(ct)[:, offs[s]:offs[s] + w],
                            scalar1=y[:, ct, i:i + 1],
                            scalar2=sc_sh[ct][:, i:i + 1],
                            op0=OP.mult, op1=OP.add)

        def gnorm(src_f, dst_f, widx, slots, awidths, tg,
                  chain_eng=None, apply_eng=None, recip_seed=True):
            var, meanp = gnorm_stats(src_f, widx, slots, tg)
            y, sc_sh = gnorm_chain(var, meanp, widx, slots, tg,
                                   chain_eng or nc.vector, recip_seed)
            gnorm_apply(src_f, dst_f, y, sc_sh, slots, awidths,
                        apply_eng or (nc.vector if widx == 0 else nc.gpsimd))
            return y, sc_sh



        # ---------- phase 2: q,k projections (slot-aligned chunks so each
        # slot's qZ depends only on its own gnorm pair; pads never projected) --
        def qkproj_chunk(o, w, on_act=False):
            for mt in range(4):
                ps = psP.tile([128, 512], F32, name="psp", tag="psp")
                for ktc in range(2):
                    mm(ps[:, :w], wqkt[:, ktc, 128 * mt:128 * mt + 128],
                       xn[ktc][:, o:o + w], start=(ktc == 0), stop=(ktc == 1))
                if mt < 2:  # q: add bias (k bias cancels in softmax)
                    if on_act:
                        nc.scalar.activation(out=qt_[mt][:, o:o + w], in_=ps[:, :w],
                                             func=AF.Identity, bias=qb[mt])
                    else:
                        nc.vector.tensor_scalar_add(qt_[mt][:, o:o + w], ps[:, :w], qb[mt])
                else:
                    if on_act:
                        nc.scalar.activation(out=kt_[mt - 2][:, o:o + w], in_=ps[:, :w],
                                             func=AF.Copy)
                    else:
                        nc.vector.tensor_copy(kt_[mt - 2][:, o:o + w], ps[:, :w])

        def qkproj_slot(s, on_act=False):
            qkproj_chunk(int(offs[s]), int(Ms[s]), on_act)

        vri = {}
        idx = 0
        for s in range(NS):
            for kt in range(nkt[s]):
                vri[(s, kt)] = idx; idx += 1
        def vproj(s, kt, on_act=False):
            mkt = min(128, Ms[s] - 128 * kt)
            ko = offs[s] + 128 * kt
            vb = 264 * vri[(s, kt)]
            ps = psP.tile([128, 512], F32, name="psp", tag="psp")
            for ctc in range(2):
                mm(ps[:mkt, :264], xn[ctc][:, ko:ko + mkt], wvot[:, ctc],
                   start=(ctc == 0), stop=(ctc == 1))
            if on_act:
                nc.scalar.activation(out=vr[:mkt, vb:vb + 264], in_=ps[:mkt, :264],
                                     func=AF.Copy)
            else:
                nc.vector.tensor_copy(vr[:mkt, vb:vb + 264], ps[:mkt, :264])
            # sumexp ones-column = 1/SCX (ctx fp8 prescale rides the ratio)
            ones = vr[:mkt, vb:vb + 264].rearrange("p (h c) -> p h c", h=8, c=33)[:, :, 32:33].squeeze(2)
            nc.gpsimd.memset(ones, 1.0 / SCX)

        # ---------- attention, software-pipelined over (slot, qtile) ----------
        def qz_slot(s, on_dve):
            M = Ms[s]
            eng = nc.vector if on_dve else nc.gpsimd
            for h in range(8):
                hp = 32 * (h % 4)
                eng.tensor_copy(qZ[hp:hp + 32, h, :M],
                                qt_[h // 4][hp:hp + 32, offs[s]:offs[s] + M])

        def attn_front(s, qi):
            """scores + exp for all key tiles of (s, qi); returns state."""
            M = Ms[s]; nv = ns_valid[s]
            qo = 128 * qi
            qc = min(128, nv - qo)
            pts = []
            for kt in range(nkt[s]):
                mkt = min(128, M - 128 * kt)
                ko = offs[s] + 128 * kt
                st = psS.tile([128, 1024], F32, name="st", tag="st")
                for h in range(8):
                    mm(st[:mkt, 128 * h:128 * h + qc],
                       kt_[h // 4][:, ko:ko + mkt], qZ[:, h, qo:qo + qc],
                       start=True, stop=True)
                pt = ptp.tile([128, 1024], BF16, name="pt", tag="pt")
                stv = st[:mkt, :].rearrange("p (h c) -> p h c", h=8, c=128)[:, :, :qc]
                ptv = pt[:mkt, :].rearrange("p (h c) -> p h c", h=8, c=128)[:, :, :qc]
                nc.scalar.activation(out=ptv, in_=stv, func=AF.Exp,
                                     bias=km[kmi[(s, kt)]][:mkt], scale=SC)
                pts.append(pt)
            return pts

        def attn_back(s, qi, pts):
            """PV + divide + transpose + ctxt store for (s, qi)."""
            M = Ms[s]; nv = ns_valid[s]
            qo = 128 * qi
            qc = min(128, nv - qo)
            qbase = offs[s] + qo
            # cs ([128,264] f32) + tp ([128,2,128] bf16 via bitcast) share
            # one PSUM bank: 1056B + 512B < 2KB
            csbank = psC.tile([128, 392], F32, name="csbank", tag="cs")
            cs = csbank[:, 0:264]
            tp = csbank[:, 264:392].bitcast(BF16).rearrange(
                "p (t c) -> p t c", t=2, c=128)
            for kt in range(nkt[s]):
                mkt = min(128, M - 128 * kt)
                vb = 264 * vri[(s, kt)]
                last = kt == nkt[s] - 1
                for h in range(8):
                    mm(cs[:qc, 33 * h:33 * h + 33],
                       pts[kt][:mkt, 128 * h:128 * h + qc],
                       vr[:mkt, vb + 33 * h:vb + 33 * h + 33],
                       start=(kt == 0), stop=last)
            # rec = SCX / sumexp ; ctxT = cs * rec (broadcast over 33-blocks)
            rec = stat.tile([128, 8], F32, name="rec", tag="rec")
            den = cs[:qc, :].rearrange("p (h c) -> p h c", h=8, c=33)[:, :, 32:33].squeeze(2)
            nc.vector.reciprocal_approx_fast(out=rec[:qc, :], in_=den)
            ctxTs = ctxp.tile([128, 256], BF16, name="ctxTs", tag="ctxTs")
            csv = cs[:qc, :].rearrange("p (h c) -> p h c", h=8, c=33)[:, :, 0:32]
            ctv = ctxTs[:qc, :].rearrange("p (h c) -> p h c", h=8, c=32)
            rv = rec[:qc, :].unsqueeze(2).broadcast_to([qc, 8, 32])
            nc.vector.tensor_tensor(out=ctv, in0=csv, in1=rv, op=OP.mult)
            for ctc in range(2):
                nc.tensor.transpose(tp[:, ctc, :qc], ctxTs[:qc, 128 * ctc:128 * ctc + 128],
                                    ident[:qc, :qc])
            nc.vector.tensor_copy(ctxt[:, :, qbase:qbase + qc], tp[:, :, :qc])

        # ---------- out_proj + residual (valid width only; pads stay 0) ----
        def outproj_slot(s):
            nv = ns_valid[s]
            o = offs[s]
            for ctc in range(2):
                ps = psP.tile([128, 512], F32, name="psp", tag="psp")
                for cw in range(0, nv, 256):  # DoubleRow rhs free = 2*w <= 512
                    w = min(256, nv - cw)
                    mm(ps[:, cw:cw + w], wot[:, ctc], ctxt[:, :, o + cw:o + cw + w],
                       start=True, stop=True, perf_mode=PM.DoubleRow)
                nc.vector.scalar_tensor_tensor(
                    out=x2[ctc][:, o:o + nv], in0=ps[:, :nv], scalar=1.0 / (SCX * SO),
                    in1=xt[ctc][:, o:o + nv], op0=OP.mult, op1=OP.add)

        # ---------- FFN, two stages for slot-level staggering ----------
        def ffn1_slot(s, pair_gelu):
            nv = ns_valid[s]
            o = offs[s]
            hg = [hgp.tile([128, 2, MMAX], FP8, name=f"hg{p}", tag=f"hg{p}") for p in range(4)]
            for p in range(4):
                if pair_gelu:
                    # one [128,1024] psum holds the mt-pair; one Gelu covers both
                    ps = psS.tile([128, 1024], F32, name="st", tag="st")
                    for half in range(2):
                        mt = 2 * p + half
                        for cw in range(0, nv, 256):
                            w = min(256, nv - cw)
                            mm(ps[:, 512 * half + cw:512 * half + cw + w],
                               w1[:, :, 128 * mt:128 * mt + 128],
                               xn2[:, :, o + cw:o + cw + w],
                               start=True, stop=True, perf_mode=PM.DoubleRow)
                    psv = ps[:, :].rearrange("p (t c) -> p t c", t=2, c=512)[:, :, :nv]
                    nc.scalar.activation(out=hg[p][:, :, :nv], in_=psv,
                                         func=AF.Gelu, scale=1.0 / S1)
                else:
                    for half in range(2):
                        mt = 2 * p + half
                        ps = psP.tile([128, 512], F32, name="psp", tag="psp")
                        for cw in range(0, nv, 256):
                            w = min(256, nv - cw)
                            mm(ps[:, cw:cw + w], w1[:, :, 128 * mt:128 * mt + 128],
                               xn2[:, :, o + cw:o + cw + w],
                               start=True, stop=True, perf_mode=PM.DoubleRow)
                        nc.scalar.activation(out=hg[p][:, half, :nv], in_=ps[:, :nv],
                                             func=AF.Gelu, bias=fb1[mt], scale=1.0 / S1)
            return hg

        def ffn2_slot(s, hg):
            nv = ns_valid[s]
            o = offs[s]
            for ctc in range(2):
                ps2 = psP.tile([128, 512], F32, name="psp", tag="psp")
                for cw in range(0, nv, 256):
                    w = min(256, nv - cw)
                    for p in range(4):
                        mm(ps2[:, cw:cw + w], w2t[:, 4 * ctc + p], hg[p][:, :, cw:cw + w],
                           start=(p == 0), stop=(p == 3), perf_mode=PM.DoubleRow)
                nc.vector.scalar_tensor_tensor(
                    out=out_t[ctc][:, o:o + nv], in0=ps2[:, :nv], scalar=1.0 / S2,
                    in1=x2[ctc][:, o:o + nv], op0=OP.mult, op1=OP.add)

        # ---------- main pipelined emission ----------
        # attention staggered by one (s, qi) item: scores/exp of item i+1 sit
        # ahead of PV(i) in the PE queue, so PE never head-of-line blocks on
        # an exp. FFN staggered by one slot for the same reason. ACT stream
        # stays [exps..., gelus...] to avoid 1283ns table reloads.
        # startup: absolute minimum before the first scores/exp of slot 0 —
        # gnorm1 pair {s0,s1}, projections of s0/s1, qZ(0), vproj(0). The
        # other pairs are emitted inside the first loop iterations and
        # pipeline their (stats -> rsqrt chain -> apply -> proj) latency
        # behind the running exp stream.
        xn_dst = lambda ct, s, w: xn[ct][:, offs[s]:offs[s] + w]
        xt_src = lambda ct: xt[ct]
        gnorm(xt_src, xn_dst, 0, [0, 1], {s: Ms[s] for s in range(NS)}, "g1p0")
        qkproj_slot(0, on_act=True)
        qz_slot(0, True)
        qkproj_slot(1, on_act=True)
        for kt in range(nkt[0]):
            vproj(0, kt, on_act=True)
        qz_slot(1, True)
        for kt in range(nkt[1]):
            vproj(1, kt, on_act=True)
        def deferred_startup(i):
            if i in (0, 1, 2):
                p = i + 1
                gnorm(xt_src, xn_dst, 0, [2 * p, 2 * p + 1],
                      {s: Ms[s] for s in range(NS)}, f"g1p{p}")
                qkproj_slot(2 * p)
                qkproj_slot(2 * p + 1)

        items = [(s, qi) for s in range(NS) for qi in range(math.ceil(ns_valid[s] / 128))]
        DEPTH = 2
        fronts = {}
        g2_half = {}

        def do_back(j):
            sj, qj = items[j]
            attn_back(sj, qj, fronts.pop(j))
            if qj == math.ceil(ns_valid[sj] / 128) - 1:
                outproj_slot(sj)
                if sj == 3:
                    g2_half[0] = gnorm(
                        lambda ct: x2[ct],
                        lambda ct, s2, w: xn2[:, ct, offs[s2]:offs[s2] + w],
                        1, [0, 1, 2, 3], {t: ns_valid[t] for t in range(NS)}, "g2a",
                        chain_eng=nc.gpsimd, recip_seed=False)

        for i, (s, qi) in enumerate(items):
            deferred_startup(i)
            if qi == 0 and s + 2 < NS:
                qz_slot(s + 2, False)
                for kt in range(nkt[s + 2]):
                    vproj(s + 2, kt)
            fronts[i] = attn_front(s, qi)
            if i >= DEPTH:
                do_back(i - DEPTH)
        for j in range(len(items) - DEPTH, len(items)):
            do_back(j)
        gnorm(lambda ct: x2[ct],
              lambda ct, s2, w: xn2[:, ct, offs[s2]:offs[s2] + w],
              1, [4, 5, 6, 7], {t: ns_valid[t] for t in range(NS)}, "g2b",
              chain_eng=nc.gpsimd, recip_seed=False)
        hgprev = None
        for s in range(NS):
            hgnew = ffn1_slot(s, pair_gelu)
            if hgprev is not None:
                ffn2_slot(s - 1, hgprev)
            hgprev = hgnew
            if s == NS // 2:
                for ctc in range(2):
                    nc.sync.dma_start(out=d_ot[ctc][:, 0:offs[NS // 2]],
                                      in_=out_t[ctc][:, 0:offs[NS // 2]])
        ffn2_slot(NS - 1, hgprev)
        for ctc in range(2):
            nc.sync.dma_start(out=d_ot[ctc][:, offs[NS // 2]:R],
                              in_=out_t[ctc][:, offs[NS // 2]:R])
    return nc


_CACHE = {}


def _prepare(inputs):
    x = np.asarray(inputs["x"], np.float32)
    batch = np.asarray(inputs["batch"]).astype(np.int64)
    counts, starts, slot_graph, Ms, offs, Rtot, R = _plan(batch)
    NS = len(Ms)
    nkt = [math.ceil(m / 128) for m in Ms]
    NKT = sum(nkt)

    in_proj_w = np.asarray(inputs["in_proj_w"], np.float32)
    in_proj_b = np.asarray(inputs["in_proj_b"], np.float32)
    out_proj_w = np.asarray(inputs["out_proj_w"], np.float32)
    out_proj_b = np.asarray(inputs["out_proj_b"], np.float32)
    ffn_w1 = np.asarray(inputs["ffn_w1"], np.float32)
    ffn_b1 = np.asarray(inputs["ffn_b1"], np.float32)
    ffn_w2 = np.asarray(inputs["ffn_w2"], np.float32)
    ffn_b2 = np.asarray(inputs["ffn_b2"], np.float32)

    # biases folded into the residual stream (gnorm is shift-invariant):
    # x' = x + out_proj_b + wo @ v_bias + ffn_b2
    fold = out_proj_b + out_proj_w @ in_proj_b[2 * H:3 * H] + ffn_b2
    xb = x + fold[None, :]

    wqk = np.ascontiguousarray(
        in_proj_w[:2 * H].T.reshape(2, 128, 512).transpose(1, 0, 2)).astype(ml_dtypes.bfloat16)
    # wv expanded to 33-col stride with zero ones-columns
    wvT = in_proj_w[2 * H:].T.reshape(2, 128, 8, 32)
    wvo = np.zeros((2, 128, 8, 33), np.float32)
    wvo[:, :, :, :32] = wvT
    wvo = np.ascontiguousarray(
        wvo.reshape(2, 128, 264).transpose(1, 0, 2)).astype(ml_dtypes.bfloat16)
    # wo fp8 [ct_out][128, 2(plane=ct_in), 128], prescaled
    woT = (out_proj_w.T * SO).reshape(2, 128, 2, 128)   # [ct_in, part, ct_out, col]
    wo8 = np.ascontiguousarray(woT.transpose(1, 2, 0, 3)).astype(ml_dtypes.float8_e4m3)
    # w1 fp8 [128, 2, 1024]: plane = input ct
    w18 = np.ascontiguousarray((ffn_w1.T * S1).reshape(2, 128, 1024).transpose(1, 0, 2)).astype(ml_dtypes.float8_e4m3)
    # w2 fp8 [8][128, 2, 128]: idx = 4*ct_out + pair; plane i = hidden 256p+128i
    w2T = (ffn_w2.T * S2).reshape(4, 2, 128, 2, 128)    # [pair, plane, part, ct_out, col]
    # [part, idx=4*ct_out+pair, plane, col]
    w28 = np.ascontiguousarray(
        w2T.transpose(2, 3, 0, 1, 4).reshape(128, 8, 2, 128)).astype(ml_dtypes.float8_e4m3)

    qkb = in_proj_b[:2 * H].reshape(4, 128)
    fb1 = ffn_b1.reshape(8, 128)
    nwv = np.stack([np.asarray(inputs["norm1_w"], np.float32).reshape(2, 128),
                    np.asarray(inputs["norm2_w"], np.float32).reshape(2, 128)])
    nbv = np.stack([np.asarray(inputs["norm1_b"], np.float32).reshape(2, 128),
                    np.asarray(inputs["norm2_b"], np.float32).reshape(2, 128)])

    xT = xb.T  # [256, N]
    xts = np.zeros((N_CORES, 2, 128, R), np.float32)
    ga1 = np.zeros((N_CORES, 128, NS), np.float32)
    gA = np.zeros((N_CORES, 128, NS), np.float32)
    gB = np.zeros((N_CORES, 128, NS), np.float32)
    kms = np.full((N_CORES, NKT, 128), NEG, np.float32)
    ns_valid = [0] * NS
    for c in range(N_CORES):
        for s in range(NS):
            g = slot_graph[c, s]
            n = int(counts[g])
            st = int(starts[g])
            o = int(offs[s])
            ns_valid[s] = max(ns_valid[s], n)
            if n > 0:
                blk = xT[:, st:st + n]
                xts[c, 0, :, o:o + n] = blk[:128]
                xts[c, 1, :, o:o + n] = blk[128:]
            ne = max(n, 1)
            ga1[c, :, s] = Ms[s] / ne
            inv_nm1 = 1.0 / max(ne - 1, 1)
            gA[c, :, s] = Ms[s] * inv_nm1
            gB[c, :, s] = Ms[s] * (1.0 - Ms[s] / ne) * inv_nm1
            ki = sum(nkt[:s])
            for kt in range(nkt[s]):
                v = min(128, max(0, n - 128 * kt))
                kms[c, ki + kt, :v] = 0.0
    ns_valid = [int(math.ceil(v / 16) * 16) if v % 16 else v for v in ns_valid]
    ns_valid = [min(v, Ms[s]) for s, v in enumerate(ns_valid)]

    pair_gelu = bool(np.all(ffn_b1 == 0))
    key = (tuple(Ms), R, tuple(ns_valid), pair_gelu)
    if key not in _CACHE:
        nc = bacc.Bacc("TRN2", target_bir_lowering=False, debug=False,
                       num_devices=N_CORES)
        _build(nc, Ms, offs, R, ns_valid, pair_gelu)
        nc.compile()
        _CACHE[key] = nc
    nc = _CACHE[key]

    ident = np.eye(128).astype(ml_dtypes.bfloat16)
    in_maps = []
    for c in range(N_CORES):
        # norm_w folded: y = rsqrt(var / nw^2) = nw * rsqrt(var) (nw >= 0)
        nw2 = np.maximum(nwv.reshape(2, 2, 128), 1e-20) ** 2  # [phase, ct, 128]
        gparts = []
        for gsrc in (gA[c], gB[c]):           # [128, NS]
            for ph in range(2):
                for ctc in range(2):
                    gparts.append(gsrc / nw2[ph, ctc][:, None])
        cstc = np.concatenate(
            [qkb[:2].T,                       # qb (q bias only)
             fb1.T,                           # 8
             nbv.reshape(4, 128).T,           # 4
             ga1[c]] + gparts +               # NS + 8*NS
            [kms[c].T], axis=1).astype(np.float32)
        in_maps.append({
            "xt": xts[c].astype(ml_dtypes.bfloat16),
            "wqk": wqk, "wvo": wvo, "wo": wo8, "w1": w18, "w2": w28,
            "cst": np.ascontiguousarray(cstc),
            "ident": ident,
        })

    def unpack(outs):
        out = np.empty((x.shape[0], H), np.float32)
        for c in range(N_CORES):
            ot = np.asarray(outs[c]["ot"]).astype(np.float32)  # [2, 128, R]
            full = np.concatenate([ot[0], ot[1]], axis=0)      # [256, R]
            for s in range(NS):
                g = slot_graph[c, s]
                n = int(counts[g])
                st = int(starts[g])
                o = int(offs[s])
                if n > 0:
                    out[st:st + n] = full[:, o:o + n].T
        return out

    return nc, in_maps, unpack


def kernel(**inputs):
    nc, in_maps, unpack = _prepare(inputs)
    res = run_bass_kernel_spmd(nc, in_maps, list(range(N_CORES)))
    return unpack(res.results)


def _traced_run(**inputs):
    """Cost-model timeline (single core) + warm wall-clock. Returns model ns."""
    import time
    nc, in_maps, unpack = _prepare(inputs)
    t0 = time.time()
    run_bass_kernel_spmd(nc, in_maps, list(range(N_CORES)))
    t1 = time.time()
    run_bass_kernel_spmd(nc, in_maps, list(range(N_CORES)))
    t2 = time.time()
    print(f"wall cold: {t1 - t0:.2f}s  warm: {t2 - t1:.2f}s")
    from concourse.timeline_sim import TimelineSim
    import trails.perfetto as _tp
    for _m in ("enable_explicit_ordering", "reserve_process_order",
               "reserve_thread_order", "set_process_order", "set_thread_order",
               "add_instant"):
        if not hasattr(_tp.LazyPerfetto, _m):
            setattr(_tp.LazyPerfetto, _m, lambda self, *a, **k: None)
    if not hasattr(_tp.LazyPerfetto, "add_counter"):
        def _add_counter(self, *a, **k):
            try:
                self.update_counter(*a, **k)
            except Exception:
                pass
        _tp.LazyPerfetto.add_counter = _add_counter
    tl = TimelineSim(nc, trace=True)
    total = tl.simulate()
    pf = tl.perfetto
    if callable(pf):
        pf = pf()
    if pf is not None:
        try:
            pf.save("/root/problem/tl.perfetto-trace")
        except Exception as e:
            print("perfetto dump failed:", e)
    return total


# revision 45
# speedup vs baseline: 1.1179x; 1.0029x over previous
"""Trainium2 Bass kernel for EnhancedTransformerBlock on ragged graphs.

Layout: transposed activations [channels (partitions), nodes (free)].
Sharding: 64 graphs -> 8 cores x 8 slots, assigned by size-sorted rank so
slot widths (uniform across cores, required for SPMD) hug the max count.

v2 design notes (vs the phase-batched f32r baseline):
- all matmul activations bf16; FFN + out_proj in fp8e4m3 with DoubleRow
  (contraction 256 per matmul at 0.5 cyc/row).
- scores: zero-padded per-head q replica (qZ) built once per slot with 4x-mode
  DVE copies; contraction 128 (4 heads of k x zero-trick).
- PV transposed: out [q<=128, 33] per head; col 33h+32 of vr holds 0.125 so the
  same matmul accumulates sumexp/8 (fp8 range prep for ctx).
- divide = stride-0 broadcast tensor_tensor; back-transpose on PE (identity).
- k needs no bias (cancels in softmax over keys); out_proj bias + wo@v_bias +
  ffn_b2 pre-added to x on host (GraphNorm is per-channel shift invariant).
- per-slot pipelining: attention(s) -> out_proj(s) -> gnorm2 stats(s); FFN per
  half interleaved with the other half's attention.
"""

import math
import numpy as np
import ml_dtypes

import concourse.bass as bass
import concourse.bacc as bacc
import concourse.mybir as mybir
import concourse.tile as tile
from concourse.bass_utils import run_bass_kernel_spmd
from contextlib import ExitStack

N_CORES = 8
B = 64
H = 256
NH = 8
HD = H // NH
EPS = 1e-5

F32 = mybir.dt.float32
BF16 = mybir.dt.bfloat16
FP8 = mybir.dt.float8e4
AF = mybir.ActivationFunctionType
OP = mybir.AluOpType
PM = mybir.MatmulPerfMode

NEG = -30.0          # additive key mask (pre-exp); exp(-30) == 0 in bf16
SC = 1.0 / math.sqrt(HD)
S1 = 32.0            # ffn_w1 fp8 prescale
S2 = 32.0            # ffn_w2 fp8 prescale
SO = 32.0            # out_proj_w fp8 prescale
SCX = 8.0            # ctx fp8 prescale (via 1/8 in the vr ones-column)


def _plan(batch):
    batch = np.asarray(batch).astype(np.int64)
    counts = np.bincount(batch, minlength=B).astype(np.int64)
    starts = np.concatenate([[0], np.cumsum(counts)[:-1]])
    order = np.argsort(-counts, kind="stable")  # rank -> graph id
    NS = B // N_CORES
    Ms, slot_graph = [], np.zeros((N_CORES, NS), np.int64)
    for s in range(NS):
        blk = order[N_CORES * s: N_CORES * s + N_CORES]
        m = int(max(16, math.ceil(max(1, counts[blk].max()) / 16) * 16))
        Ms.append(m)
        for c in range(N_CORES):
            slot_graph[c, s] = blk[c]
    offs = np.concatenate([[0], np.cumsum(Ms)]).astype(np.int64)
    Rtot = int(offs[-1])
    R = int(math.ceil(Rtot / 128) * 128)
    return counts, starts, slot_graph, Ms, offs, Rtot, R


def _build(nc, Ms, offs, R, ns_valid, pair_gelu):
    """ns_valid[s] = max valid node count over cores for slot s (<= Ms[s]).
    Per-core valid counts differ; we compute the slot at the max width and the
    km mask (per core) zeroes the prob rows beyond each core's own count.
    Query-side trims use ns_valid (same extent every core keeps SPMD single
    program); pads beyond ns_valid are never read back by any core."""
    NS = len(Ms)
    nkt = [math.ceil(m / 128) for m in Ms]
    NKT = sum(nkt)
    MMAX = max(Ms)

    # ---- DRAM ----
    d_xt = nc.dram_tensor("xt", [2, 128, R], BF16, kind="ExternalInput").ap()
    d_wqk = nc.dram_tensor("wqk", [128, 2, 512], BF16, kind="ExternalInput").ap()
    d_wvo = nc.dram_tensor("wvo", [128, 2, 264], BF16, kind="ExternalInput").ap()
    d_wo = nc.dram_tensor("wo", [128, 2, 2, 128], FP8, kind="ExternalInput").ap()
    d_w1 = nc.dram_tensor("w1", [128, 2, 1024], FP8, kind="ExternalInput").ap()
    d_w2 = nc.dram_tensor("w2", [128, 8, 2, 128], FP8, kind="ExternalInput").ap()
    # packed per-partition constants:
    # [qb(2) fb1(8) nb(4) ga1(NS) gAp(4*NS) gBp(4*NS) km(NKT)]
    NCST = 14 + 9 * NS + NKT
    d_cst = nc.dram_tensor("cst", [128, NCST], F32, kind="ExternalInput").ap()
    d_id = nc.dram_tensor("ident", [128, 128], BF16, kind="ExternalInput").ap()
    d_ot = nc.dram_tensor("ot", [2, 128, R], BF16, kind="ExternalOutput").ap()

    mm = nc.tensor.matmul

    with tile.TileContext(nc) as tc, ExitStack() as ctx:
        pers = ctx.enter_context(tc.tile_pool(name="pers", bufs=1))
        ptp = ctx.enter_context(tc.tile_pool(name="ptp", bufs=12))
        hgp = ctx.enter_context(tc.tile_pool(name="hgp", bufs=2))
        stat = ctx.enter_context(tc.tile_pool(name="stat", bufs=4))
        ctxp = ctx.enter_context(tc.tile_pool(name="ctxp", bufs=3))
        psP = ctx.enter_context(tc.tile_pool(name="psP", bufs=2, space="PSUM"))
        psS = ctx.enter_context(tc.tile_pool(name="psS", bufs=2, space="PSUM"))
        psC = ctx.enter_context(tc.tile_pool(name="psC", bufs=2, space="PSUM"))

        # ---- persistent SBUF tiles ----
        cst = pers.tile([128, NCST], F32, name="cst", tag="cst")
        nc.sync.dma_start(out=cst, in_=d_cst)
        co = 0
        def cslice(n):
            nonlocal co
            a = cst[:, co:co + n]; co += n
            return a
        qb = [cslice(1) for _ in range(2)]
        fb1 = [cslice(1) for _ in range(8)]
        nb = [[cslice(1) for _ in range(2)] for _ in range(2)]
        ga1 = cslice(NS)
        gAp = [[cslice(NS) for _ in range(2)] for _ in range(2)]
        gBp = [[cslice(NS) for _ in range(2)] for _ in range(2)]
        km = [cslice(1) for _ in range(NKT)]
        kmi = {}
        idx = 0
        for s in range(NS):
            for kt in range(nkt[s]):
                kmi[(s, kt)] = idx; idx += 1

        # Few LARGE DMAs, critical-first: each DMACopy pays ~1.4us of fixed
        # HWDGE/sem overhead, so slot-granular loads serialize the startup.
        q_off = int(offs[NS // 4])
        half_off = int(offs[NS // 2])
        ident = pers.tile([128, 128], BF16, name="ident", tag="ident")
        nc.sync.dma_start(out=ident, in_=d_id)
        xt = [pers.tile([128, R], BF16, name=f"xt{i}", tag=f"xt{i}") for i in range(2)]
        for ct in range(2):
            nc.sync.dma_start(out=xt[ct][:, 0:q_off], in_=d_xt[ct][:, 0:q_off])
        wqkt = pers.tile([128, 2, 512], BF16, name="wqkt", tag="wqkt")
        nc.sync.dma_start(out=wqkt, in_=d_wqk)
        wvot = pers.tile([128, 2, 264], BF16, name="wvot", tag="wvot")
        nc.sync.dma_start(out=wvot, in_=d_wvo)
        for ct in range(2):
            nc.sync.dma_start(out=xt[ct][:, q_off:half_off], in_=d_xt[ct][:, q_off:half_off])
        for ct in range(2):
            nc.sync.dma_start(out=xt[ct][:, half_off:R], in_=d_xt[ct][:, half_off:R])
        wot = pers.tile([128, 2, 2, 128], FP8, name="wot", tag="wot")
        nc.sync.dma_start(out=wot, in_=d_wo)
        w1 = pers.tile([128, 2, 1024], FP8, name="w1", tag="w1")
        nc.sync.dma_start(out=w1, in_=d_w1)
        w2t = pers.tile([128, 8, 2, 128], FP8, name="w2t", tag="w2t")
        nc.sync.dma_start(out=w2t, in_=d_w2)

        # PE p-state warmup: ~24 dep-free matmuls keep the PE continuously
        # busy from t~1us so real matmuls start at the fast clock.
        for _ in range(36):
            wps = psP.tile([128, 512], F32, name="psp", tag="psp")
            mm(wps[:, :128], ident, ident, start=True, stop=True)

        xn = [pers.tile([128, R], BF16, name=f"xn{i}", tag=f"xn{i}") for i in range(2)]
        qt_ = [pers.tile([128, R], BF16, name=f"q{i}", tag=f"q{i}") for i in range(2)]
        kt_ = [pers.tile([128, R], BF16, name=f"k{i}", tag=f"k{i}") for i in range(2)]
        qZ = pers.tile([128, 8, MMAX], BF16, name="qZ", tag="qZ")
        nc.gpsimd.memset(qZ, 0.0)  # persistent zeros; head h only ever writes rows 32*(h%4)
        vr = pers.tile([128, 264 * NKT], BF16, name="vr", tag="vr")
        ctxt = pers.tile([128, 2, R], FP8, name="ctxt", tag="ctxt")
        x2 = [pers.tile([128, R], BF16, name=f"x2{i}", tag=f"x2{i}") for i in range(2)]
        for ct in range(2):
            nc.gpsimd.memset(x2[ct], 0.0)  # pads must stay 0 for gnorm2 stats
        xn2 = pers.tile([128, 2, R], FP8, name="xn2", tag="xn2")
        out_t = [pers.tile([128, R], BF16, name=f"ot{i}", tag=f"ot{i}") for i in range(2)]

        # ---------- GraphNorm stats+apply ----------
        # rstd = rsqrt(var) via DVE reciprocal seed + Newton (keeps the ACT
        # engine free of Ln/Exp and their 1283ns table loads; eps ~ 1e-5 on a
        # ~1.0 std is far below the fp8/bf16 noise floor, dropped).
        def gnorm_stats(src_f, widx, slots, tg):
            """bn_stats (DVE only) + var/mean prep; returns (var, meanp)."""
            nsl = len(slots)
            c0 = slots[0]
            var = stat.tile([128, 2, nsl], F32, name="var", tag=f"var{tg}")
            meanp = stat.tile([128, 2, nsl], F32, name="meanp", tag=f"meanp{tg}")
            for ct in range(2):
                mv = stat.tile([128, 2, nsl], F32, name="mv", tag=f"mv{tg}{ct}")
                for i, s in enumerate(slots):
                    st6 = stat.tile([128, 6], F32, name="st6", tag="st6")
                    nc.vector.bn_stats(out=st6, in_=src_f(ct)[:, offs[s]:offs[s] + Ms[s]])
                    nc.vector.bn_aggr(out=mv[:, :, i:i + 1], in_=st6)
                mean_r = mv[:, 0:1, :].squeeze(1)
                var_r = mv[:, 1:2, :].squeeze(1)
                m2 = stat.tile([128, nsl], F32, name="m2", tag="m2")
                nc.vector.tensor_mul(m2, mean_r, mean_r)
                v1 = stat.tile([128, nsl], F32, name="v1", tag="v1")
                nc.vector.tensor_mul(v1, var_r, gAp[widx][ct][:, c0:c0 + nsl])
                nc.vector.tensor_mul(var[:, ct, :], m2, gBp[widx][ct][:, c0:c0 + nsl])
                nc.vector.tensor_add(var[:, ct, :], var[:, ct, :], v1)
                nc.vector.tensor_mul(meanp[:, ct, :], mean_r, ga1[:, c0:c0 + nsl])
            return var, meanp

        def gnorm_chain(var, meanp, widx, slots, tg, eng, recip_seed):
            """rsqrt chain -> (y == scale, per-ct shift). Runs on `eng` so the
            ~6us of serial hop latency doesn't head-of-line block DVE."""
            nsl = len(slots)
            vv = var[:, :, :]
            y = stat.tile([128, 2, nsl], F32, name="y", tag=f"y{tg}")
            t = stat.tile([128, 2, nsl], F32, name="t", tag=f"t{tg}")
            if recip_seed:
                nc.vector.reciprocal_approx_fast(out=y, in_=vv)
                eng.tensor_scalar(out=y, in0=y, scalar1=0.5, scalar2=0.5,
                                  op0=OP.mult, op1=OP.add)
                iters = 1
            else:  # linear seed 1.5 - 0.5v (fine for var in [0.5, 1.6])
                eng.tensor_scalar(out=y, in0=vv, scalar1=-0.5, scalar2=1.5,
                                  op0=OP.mult, op1=OP.add)
                iters = 2
            for _ in range(iters):
                eng.tensor_mul(t, vv, y)
                eng.tensor_mul(t, t, y)
                eng.tensor_scalar(out=t, in0=t, scalar1=-0.5, scalar2=1.5,
                                  op0=OP.mult, op1=OP.add)
                eng.tensor_mul(y, y, t)
            sc_sh = []
            for ct in range(2):
                shift = stat.tile([128, nsl], F32, name="shift", tag=f"shift{tg}{ct}")
                eng.tensor_mul(shift, meanp[:, ct, :], y[:, ct, :])
                eng.tensor_scalar(
                    out=shift, in0=shift, scalar1=-1.0, scalar2=nb[widx][ct],
                    op0=OP.mult, op1=OP.add)
                sc_sh.append(shift)
            return y, sc_sh

        def gnorm_apply(src_f, dst_f, y, sc_sh, slots, awidths, eng, sel=None):
            for i, s in enumerate(slots):
                if sel is not None and s not in sel:
                    continue
                for ct in range(2):
                    w = awidths[s]
                    if eng is nc.scalar:  # ACT: out = Identity(scale*in + bias)
                        nc.scalar.activation(
                            out=dst_f(ct, s, w),
                            in_=src_f(ct)[:, offs[s]:offs[s] + w],
                            func=AF.Identity,
                            bias=sc_sh[ct][:, i:i + 1],
                            scale=y[:, ct, i:i + 1])
                    else:
                        eng.tensor_scalar(
                            out=dst_f(ct, s, w),
                            in0=src_f(ct)[:, offs[s]:offs[s] + w],
                            scalar1=y[:, ct, i:i + 1],
                            scalar2=sc_sh[ct][:, i:i + 1],
                            op0=OP.mult, op1=OP.add)

        def gnorm(src_f, dst_f, widx, slots, awidths, tg,
                  chain_eng=None, apply_eng=None, recip_seed=True):
            var, meanp = gnorm_stats(src_f, widx, slots, tg)
            y, sc_sh = gnorm_chain(var, meanp, widx, slots, tg,
                                   chain_eng or nc.vector, recip_seed)
            gnorm_apply(src_f, dst_f, y, sc_sh, slots, awidths,
                        apply_eng or (nc.vector if widx == 0 else nc.gpsimd))
            return y, sc_sh



        # ---------- phase 2: q,k projections (slot-aligned chunks so each
        # slot's qZ depends only on its own gnorm pair; pads never projected) --
        def qkproj_chunk(o, w, on_act=False):
            for mt in range(4):
                ps = psP.tile([128, 512], F32, name="psp", tag="psp")
                for ktc in range(2):
                    mm(ps[:, :w], wqkt[:, ktc, 128 * mt:128 * mt + 128],
                       xn[ktc][:, o:o + w], start=(ktc == 0), stop=(ktc == 1))
                if mt < 2:  # q: add bias (k bias cancels in softmax)
                    if on_act:
                        nc.scalar.activation(out=qt_[mt][:, o:o + w], in_=ps[:, :w],
                                             func=AF.Identity, bias=qb[mt])
                    else:
                        nc.vector.tensor_scalar_add(qt_[mt][:, o:o + w], ps[:, :w], qb[mt])
                else:
                    if on_act:
                        nc.scalar.activation(out=kt_[mt - 2][:, o:o + w], in_=ps[:, :w],
                                             func=AF.Copy)
                    else:
                        nc.vector.tensor_copy(kt_[mt - 2][:, o:o + w], ps[:, :w])

        def qkproj_slot(s, on_act=False):
            qkproj_chunk(int(offs[s]), int(Ms[s]), on_act)

        vri = {}
        idx = 0
        for s in range(NS):
            for kt in range(nkt[s]):
                vri[(s, kt)] = idx; idx += 1
        def vproj(s, kt, on_act=False):
            mkt = min(128, Ms[s] - 128 * kt)
            ko = offs[s] + 128 * kt
            vb = 264 * vri[(s, kt)]
            ps = psP.tile([128, 512], F32, name="psp", tag="psp")
            for ctc in range(2):
                mm(ps[:mkt, :264], xn[ctc][:, ko:ko + mkt], wvot[:, ctc],
                   start=(ctc == 0), stop=(ctc == 1))
            if on_act:
                nc.scalar.activation(out=vr[:mkt, vb:vb + 264], in_=ps[:mkt, :264],
                                     func=AF.Copy)
            else:
                nc.vector.tensor_copy(vr[:mkt, vb:vb + 264], ps[:mkt, :264])
            # sumexp ones-column = 1/SCX (ctx fp8 prescale rides the ratio)
            ones = vr[:mkt, vb:vb + 264].rearrange("p (h c) -> p h c", h=8, c=33)[:, :, 32:33].squeeze(2)
            nc.gpsimd.memset(ones, 1.0 / SCX)

        # ---------- attention, software-pipelined over (slot, qtile) ----------
        def qz_slot(s, on_dve):
            M = Ms[s]
            eng = nc.vector if on_dve else nc.gpsimd
            for h in range(8):
                hp = 32 * (h % 4)
                eng.tensor_copy(qZ[hp:hp + 32, h, :M],
                                qt_[h // 4][hp:hp + 32, offs[s]:offs[s] + M])

        def attn_front(s, qi):
            """scores + exp for all key tiles of (s, qi); returns state."""
            M = Ms[s]; nv = ns_valid[s]
            qo = 128 * qi
            qc = min(128, nv - qo)
            pts = []
            for kt in range(nkt[s]):
                mkt = min(128, M - 128 * kt)
                ko = offs[s] + 128 * kt
                st = psS.tile([128, 1024], F32, name="st", tag="st")
                for h in range(8):
                    mm(st[:mkt, 128 * h:128 * h + qc],
                       kt_[h // 4][:, ko:ko + mkt], qZ[:, h, qo:qo + qc],
                       start=True, stop=True)
                pt = ptp.tile([128, 1024], BF16, name="pt", tag="pt")
                stv = st[:mkt, :].rearrange("p (h c) -> p h c", h=8, c=128)[:, :, :qc]
                ptv = pt[:mkt, :].rearrange("p (h c) -> p h c", h=8, c=128)[:, :, :qc]
                nc.scalar.activation(out=ptv, in_=stv, func=AF.Exp,
                                     bias=km[kmi[(s, kt)]][:mkt], scale=SC)
                pts.append(pt)
            return pts

        def attn_back(s, qi, pts):
            """PV + divide + transpose + ctxt store for (s, qi)."""
            M = Ms[s]; nv = ns_valid[s]
            qo = 128 * qi
            qc = min(128, nv - qo)
            qbase = offs[s] + qo
            # cs ([128,264] f32) + tp ([128,2,128] bf16 via bitcast) share
            # one PSUM bank: 1056B + 512B < 2KB
            csbank = psC.tile([128, 392], F32, name="csbank", tag="cs")
            cs = csbank[:, 0:264]
            tp = csbank[:, 264:392].bitcast(BF16).rearrange(
                "p (t c) -> p t c", t=2, c=128)
            for kt in range(nkt[s]):
                mkt = min(128, M - 128 * kt)
                vb = 264 * vri[(s, kt)]
                last = kt == nkt[s] - 1
                for h in range(8):
                    mm(cs[:qc, 33 * h:33 * h + 33],
                       pts[kt][:mkt, 128 * h:128 * h + qc],
                       vr[:mkt, vb + 33 * h:vb + 33 * h + 33],
                       start=(kt == 0), stop=last)
            # rec = SCX / sumexp ; ctxT = cs * rec (broadcast over 33-blocks)
            rec = stat.tile([128, 8], F32, name="rec", tag="rec")
            den = cs[:qc, :].rearrange("p (h c) -> p h c", h=8, c=33)[:, :, 32:33].squeeze(2)
            nc.vector.reciprocal_approx_fast(out=rec[:qc, :], in_=den)
            ctxTs = ctxp.tile([128, 256], BF16, name="ctxTs", tag="ctxTs")
            csv = cs[:qc, :].rearrange("p (h c) -> p h c", h=8, c=33)[:, :, 0:32]
            ctv = ctxTs[:qc, :].rearrange("p (h c) -> p h c", h=8, c=32)
            rv = rec[:qc, :].unsqueeze(2).broadcast_to([qc, 8, 32])
            nc.vector.tensor_tensor(out=ctv, in0=csv, in1=rv, op=OP.mult)
            for ctc in range(2):
                nc.tensor.transpose(tp[:, ctc, :qc], ctxTs[:qc, 128 * ctc:128 * ctc + 128],
                                    ident[:qc, :qc])
            nc.vector.tensor_copy(ctxt[:, :, qbase:qbase + qc], tp[:, :, :qc])

        # ---------- out_proj + residual (valid width only; pads stay 0) ----
        def outproj_slot(s):
            nv = ns_valid[s]
            o = offs[s]
            for ctc in range(2):
                ps = psP.tile([128, 512], F32, name="psp", tag="psp")
                for cw in range(0, nv, 256):  # DoubleRow rhs free = 2*w <= 512
                    w = min(256, nv - cw)
                    mm(ps[:, cw:cw + w], wot[:, ctc], ctxt[:, :, o + cw:o + cw + w],
                       start=True, stop=True, perf_mode=PM.DoubleRow)
                nc.vector.scalar_tensor_tensor(
                    out=x2[ctc][:, o:o + nv], in0=ps[:, :nv], scalar=1.0 / (SCX * SO),
                    in1=xt[ctc][:, o:o + nv], op0=OP.mult, op1=OP.add)

        # ---------- FFN, two stages for slot-level staggering ----------
        def ffn1_slot(s, pair_gelu):
            nv = ns_valid[s]
            o = offs[s]
            hg = [hgp.tile([128, 2, MMAX], FP8, name=f"hg{p}", tag=f"hg{p}") for p in range(4)]
            for p in range(4):
                if pair_gelu:
                    # one [128,1024] psum holds the mt-pair; one Gelu covers both
                    ps = psS.tile([128, 1024], F32, name="st", tag="st")
                    for half in range(2):
                        mt = 2 * p + half
                        for cw in range(0, nv, 256):
                            w = min(256, nv - cw)
                            mm(ps[:, 512 * half + cw:512 * half + cw + w],
                               w1[:, :, 128 * mt:128 * mt + 128],
                               xn2[:, :, o + cw:o + cw + w],
                               start=True, stop=True, perf_mode=PM.DoubleRow)
                    psv = ps[:, :].rearrange("p (t c) -> p t c", t=2, c=512)[:, :, :nv]
                    nc.scalar.activation(out=hg[p][:, :, :nv], in_=psv,
                                         func=AF.Gelu, scale=1.0 / S1)
                else:
                    for half in range(2):
                        mt = 2 * p + half
                        ps = psP.tile([128, 512], F32, name="psp", tag="psp")
                        for cw in range(0, nv, 256):
                            w = min(256, nv - cw)
                            mm(ps[:, cw:cw + w], w1[:, :, 128 * mt:128 * mt + 128],
                               xn2[:, :, o + cw:o + cw + w],
                               start=True, stop=True, perf_mode=PM.DoubleRow)
                        nc.scalar.activation(out=hg[p][:, half, :nv], in_=ps[:, :nv],
                                             func=AF.Gelu, bias=fb1[mt], scale=1.0 / S1)
            return hg

        def ffn2_slot(s, hg):
            nv = ns_valid[s]
            o = offs[s]
            for ctc in range(2):
                ps2 = psP.tile([128, 512], F32, name="psp", tag="psp")
                for cw in range(0, nv, 256):
                    w = min(256, nv - cw)
                    for p in range(4):
                        mm(ps2[:, cw:cw + w], w2t[:, 4 * ctc + p], hg[p][:, :, cw:cw + w],
                           start=(p == 0), stop=(p == 3), perf_mode=PM.DoubleRow)
                nc.vector.scalar_tensor_tensor(
                    out=out_t[ctc][:, o:o + nv], in0=ps2[:, :nv], scalar=1.0 / S2,
                    in1=x2[ctc][:, o:o + nv], op0=OP.mult, op1=OP.add)

        # ---------- main pipelined emission ----------
        # attention staggered by one (s, qi) item: scores/exp of item i+1 sit
        # ahead of PV(i) in the PE queue, so PE never head-of-line blocks on
        # an exp. FFN staggered by one slot for the same reason. ACT stream
        # stays [exps..., gelus...] to avoid 1283ns table reloads.
        # startup: absolute minimum before the first scores/exp of slot 0 —
        # gnorm1 pair {s0,s1}, projections of s0/s1, qZ(0), vproj(0). The
        # other pairs are emitted inside the first loop iterations and
        # pipeline their (stats -> rsqrt chain -> apply -> proj) latency
        # behind the running exp stream.
        xn_dst = lambda ct, s, w: xn[ct][:, offs[s]:offs[s] + w]
        xt_src = lambda ct: xt[ct]
        MW = {s: Ms[s] for s in range(NS)}
        # pair 0 start path: stats(DVE) -> chain(Pool) -> applies+proj(ACT) ->
        # qZ(DVE). The ACT engine is idle until the first exp, and the Pool
        # chain keeps DVE free for the stats of pairs 1-3 (DMA-gated).
        v0, m0 = gnorm_stats(xt_src, 0, [0, 1], "g1p0")
        y0_, s0_ = gnorm_chain(v0, m0, 0, [0, 1], "g1p0", nc.gpsimd, recip_seed=False)
        gnorm_apply(xt_src, xn_dst, y0_, s0_, [0, 1], MW, nc.scalar)
        qkproj_slot(0, on_act=True)
        qz_slot(0, True)
        qkproj_slot(1, on_act=True)
        for kt in range(nkt[0]):
            vproj(0, kt, on_act=True)
        qz_slot(1, True)
        for kt in range(nkt[1]):
            vproj(1, kt, on_act=True)
        g1ys = {0: (y0_, s0_)}
        for p in (1, 2, 3):
            vp, mp = gnorm_stats(xt_src, 0, [2 * p, 2 * p + 1], f"g1p{p}")
            g1ys[p] = gnorm_chain(vp, mp, 0, [2 * p, 2 * p + 1], f"g1p{p}",
                                  nc.gpsimd, recip_seed=False)

        items = [(s, qi) for s in range(NS) for qi in range(math.ceil(ns_valid[s] / 128))]
        DEPTH = 2
        fronts = {}

        def do_back(j):
            sj, qj = items[j]
            attn_back(sj, qj, fronts.pop(j))
            if qj == math.ceil(ns_valid[sj] / 128) - 1:
                outproj_slot(sj)
                if sj == 3:
                    gnorm(lambda ct: x2[ct],
                          lambda ct, s2, w: xn2[:, ct, offs[s2]:offs[s2] + w],
                          1, [0, 1, 2, 3], {t: ns_valid[t] for t in range(NS)}, "g2a",
                          chain_eng=nc.gpsimd, recip_seed=False)

        for i, (s, qi) in enumerate(items):
            nq_s = math.ceil(ns_valid[s] / 128)
            if qi == nq_s - 1 and s <= 2:
                # pair s+1 applies + projections on ACT at the slot boundary
                p = s + 1
                yp, sp = g1ys[p]
                gnorm_apply(xt_src, xn_dst, yp, sp, [2 * p, 2 * p + 1], MW,
                            nc.scalar)
                qkproj_slot(2 * p, on_act=True)
                qkproj_slot(2 * p + 1, on_act=True)
            if qi == 0 and 1 <= s and s + 1 < NS:
                # one slot of lead: slot s+1 qZ + vr on DVE (ready by now)
                qz_slot(s + 1, True)
                for kt in range(nkt[s + 1]):
                    vproj(s + 1, kt)
            fronts[i] = attn_front(s, qi)
            if i >= DEPTH:
                do_back(i - DEPTH)
        for j in range(len(items) - DEPTH, len(items)):
            do_back(j)
        gnorm(lambda ct: x2[ct],
              lambda ct, s2, w: xn2[:, ct, offs[s2]:offs[s2] + w],
              1, [4, 5, 6, 7], {t: ns_valid[t] for t in range(NS)}, "g2b",
              chain_eng=nc.gpsimd, recip_seed=False)
        hgprev = None
        for s in range(NS):
            hgnew = ffn1_slot(s, pair_gelu)
            if hgprev is not None:
                ffn2_slot(s - 1, hgprev)
            hgprev = hgnew
            if s == NS // 2:
                for ctc in range(2):
                    nc.sync.dma_start(out=d_ot[ctc][:, 0:offs[NS // 2]],
                                      in_=out_t[ctc][:, 0:offs[NS // 2]])
        ffn2_slot(NS - 1, hgprev)
        for ctc in range(2):
            nc.sync.dma_start(out=d_ot[ctc][:, offs[NS // 2]:R],
                              in_=out_t[ctc][:, offs[NS // 2]:R])
    return nc


_CACHE = {}


def _prepare(inputs):
    x = np.asarray(inputs["x"], np.float32)
    batch = np.asarray(inputs["batch"]).astype(np.int64)
    counts, starts, slot_graph, Ms, offs, Rtot, R = _plan(batch)
    NS = len(Ms)
    nkt = [math.ceil(m / 128) for m in Ms]
    NKT = sum(nkt)

    in_proj_w = np.asarray(inputs["in_proj_w"], np.float32)
    in_proj_b = np.asarray(inputs["in_proj_b"], np.float32)
    out_proj_w = np.asarray(inputs["out_proj_w"], np.float32)
    out_proj_b = np.asarray(inputs["out_proj_b"], np.float32)
    ffn_w1 = np.asarray(inputs["ffn_w1"], np.float32)
    ffn_b1 = np.asarray(inputs["ffn_b1"], np.float32)
    ffn_w2 = np.asarray(inputs["ffn_w2"], np.float32)
    ffn_b2 = np.asarray(inputs["ffn_b2"], np.float32)

    # biases folded into the residual stream (gnorm is shift-invariant):
    # x' = x + out_proj_b + wo @ v_bias + ffn_b2
    fold = out_proj_b + out_proj_w @ in_proj_b[2 * H:3 * H] + ffn_b2
    xb = x + fold[None, :]

    wqk = np.ascontiguousarray(
        in_proj_w[:2 * H].T.reshape(2, 128, 512).transpose(1, 0, 2)).astype(ml_dtypes.bfloat16)
    # wv expanded to 33-col stride with zero ones-columns
    wvT = in_proj_w[2 * H:].T.reshape(2, 128, 8, 32)
    wvo = np.zeros((2, 128, 8, 33), np.float32)
    wvo[:, :, :, :32] = wvT
    wvo = np.ascontiguousarray(
        wvo.reshape(2, 128, 264).transpose(1, 0, 2)).astype(ml_dtypes.bfloat16)
    # wo fp8 [ct_out][128, 2(plane=ct_in), 128], prescaled
    woT = (out_proj_w.T * SO).reshape(2, 128, 2, 128)   # [ct_in, part, ct_out, col]
    wo8 = np.ascontiguousarray(woT.transpose(1, 2, 0, 3)).astype(ml_dtypes.float8_e4m3)
    # w1 fp8 [128, 2, 1024]: plane = input ct
    w18 = np.ascontiguousarray((ffn_w1.T * S1).reshape(2, 128, 1024).transpose(1, 0, 2)).astype(ml_dtypes.float8_e4m3)
    # w2 fp8 [8][128, 2, 128]: idx = 4*ct_out + pair; plane i = hidden 256p+128i
    w2T = (ffn_w2.T * S2).reshape(4, 2, 128, 2, 128)    # [pair, plane, part, ct_out, col]
    # [part, idx=4*ct_out+pair, plane, col]
    w28 = np.ascontiguousarray(
        w2T.transpose(2, 3, 0, 1, 4).reshape(128, 8, 2, 128)).astype(ml_dtypes.float8_e4m3)

    qkb = in_proj_b[:2 * H].reshape(4, 128)
    fb1 = ffn_b1.reshape(8, 128)
    nwv = np.stack([np.asarray(inputs["norm1_w"], np.float32).reshape(2, 128),
                    np.asarray(inputs["norm2_w"], np.float32).reshape(2, 128)])
    nbv = np.stack([np.asarray(inputs["norm1_b"], np.float32).reshape(2, 128),
                    np.asarray(inputs["norm2_b"], np.float32).reshape(2, 128)])

    xT = xb.T  # [256, N]
    xts = np.zeros((N_CORES, 2, 128, R), np.float32)
    ga1 = np.zeros((N_CORES, 128, NS), np.float32)
    gA = np.zeros((N_CORES, 128, NS), np.float32)
    gB = np.zeros((N_CORES, 128, NS), np.float32)
    kms = np.full((N_CORES, NKT, 128), NEG, np.float32)
    ns_valid = [0] * NS
    for c in range(N_CORES):
        for s in range(NS):
            g = slot_graph[c, s]
            n = int(counts[g])
            st = int(starts[g])
            o = int(offs[s])
            ns_valid[s] = max(ns_valid[s], n)
            if n > 0:
                blk = xT[:, st:st + n]
                xts[c, 0, :, o:o + n] = blk[:128]
                xts[c, 1, :, o:o + n] = blk[128:]
            ne = max(n, 1)
            ga1[c, :, s] = Ms[s] / ne
            inv_nm1 = 1.0 / max(ne - 1, 1)
            gA[c, :, s] = Ms[s] * inv_nm1
            gB[c, :, s] = Ms[s] * (1.0 - Ms[s] / ne) * inv_nm1
            ki = sum(nkt[:s])
            for kt in range(nkt[s]):
                v = min(128, max(0, n - 128 * kt))
                kms[c, ki + kt, :v] = 0.0
    ns_valid = [int(math.ceil(v / 16) * 16) if v % 16 else v for v in ns_valid]
    ns_valid = [min(v, Ms[s]) for s, v in enumerate(ns_valid)]

    pair_gelu = bool(np.all(ffn_b1 == 0))
    key = (tuple(Ms), R, tuple(ns_valid), pair_gelu)
    if key not in _CACHE:
        nc = bacc.Bacc("TRN2", target_bir_lowering=False, debug=False,
                       num_devices=N_CORES)
        _build(nc, Ms, offs, R, ns_valid, pair_gelu)
        nc.compile()
        _CACHE[key] = nc
    nc = _CACHE[key]

    ident = np.eye(128).astype(ml_dtypes.bfloat16)
    in_maps = []
    for c in range(N_CORES):
        # norm_w folded: y = rsqrt(var / nw^2) = nw * rsqrt(var) (nw >= 0)
        nw2 = np.maximum(nwv.reshape(2, 2, 128), 1e-20) ** 2  # [phase, ct, 128]
        gparts = []
        for gsrc in (gA[c], gB[c]):           # [128, NS]
            for ph in range(2):
                for ctc in range(2):
                    gparts.append(gsrc / nw2[ph, ctc][:, None])
        cstc = np.concatenate(
            [qkb[:2].T,                       # qb (q bias only)
             fb1.T,                           # 8
             nbv.reshape(4, 128).T,           # 4
             ga1[c]] + gparts +               # NS + 8*NS
            [kms[c].T], axis=1).astype(np.float32)
        in_maps.append({
            "xt": xts[c].astype(ml_dtypes.bfloat16),
            "wqk": wqk, "wvo": wvo, "wo": wo8, "w1": w18, "w2": w28,
            "cst": np.ascontiguousarray(cstc),
            "ident": ident,
        })

    def unpack(outs):
        out = np.empty((x.shape[0], H), np.float32)
        for c in range(N_CORES):
            ot = np.asarray(outs[c]["ot"]).astype(np.float32)  # [2, 128, R]
            full = np.concatenate([ot[0], ot[1]], axis=0)      # [256, R]
            for s in range(NS):
                g = slot_graph[c, s]
                n = int(counts[g])
                st = int(starts[g])
                o = int(offs[s])
                if n > 0:
                    out[st:st + n] = full[:, o:o + n].T
        return out

    return nc, in_maps, unpack


def kernel(**inputs):
    nc, in_maps, unpack = _prepare(inputs)
    res = run_bass_kernel_spmd(nc, in_maps, list(range(N_CORES)))
    return unpack(res.results)


def _traced_run(**inputs):
    """Cost-model timeline (single core) + warm wall-clock. Returns model ns."""
    import time
    nc, in_maps, unpack = _prepare(inputs)
    t0 = time.time()
    run_bass_kernel_spmd(nc, in_maps, list(range(N_CORES)))
    t1 = time.time()
    run_bass_kernel_spmd(nc, in_maps, list(range(N_CORES)))
    t2 = time.time()
    print(f"wall cold: {t1 - t0:.2f}s  warm: {t2 - t1:.2f}s")
    from concourse.timeline_sim import TimelineSim
    import trails.perfetto as _tp
    for _m in ("enable_explicit_ordering", "reserve_process_order",
               "reserve_thread_order", "set_process_order", "set_thread_order",
               "add_instant"):
        if not hasattr(_tp.LazyPerfetto, _m):
            setattr(_tp.LazyPerfetto, _m, lambda self, *a, **k: None)
    if not hasattr(_tp.LazyPerfetto, "add_counter"):
        def _add_counter(self, *a, **k):
            try:
                self.update_counter(*a, **k)
            except Exception:
                pass
        _tp.LazyPerfetto.add_counter = _add_counter
    tl = TimelineSim(nc, trace=True)
    total = tl.simulate()
    pf = tl.perfetto
    if callable(pf):
        pf = pf()
    if pf is not None:
        try:
            pf.save("/root/problem/tl.perfetto-trace")
        except Exception as e:
            print("perfetto dump failed:", e)
    return total


# revision 47
# speedup vs baseline: 1.1952x; 1.0692x over previous
"""Trainium2 Bass kernel for EnhancedTransformerBlock on ragged graphs.

Layout: transposed activations [channels (partitions), nodes (free)].
Sharding: 64 graphs -> 8 cores x 8 slots, assigned by size-sorted rank so
slot widths (uniform across cores, required for SPMD) hug the max count.

v2 design notes (vs the phase-batched f32r baseline):
- all matmul activations bf16; FFN + out_proj in fp8e4m3 with DoubleRow
  (contraction 256 per matmul at 0.5 cyc/row).
- scores: zero-padded per-head q replica (qZ) built once per slot with 4x-mode
  DVE copies; contraction 128 (4 heads of k x zero-trick).
- PV transposed: out [q<=128, 33] per head; col 33h+32 of vr holds 0.125 so the
  same matmul accumulates sumexp/8 (fp8 range prep for ctx).
- divide = stride-0 broadcast tensor_tensor; back-transpose on PE (identity).
- k needs no bias (cancels in softmax over keys); out_proj bias + wo@v_bias +
  ffn_b2 pre-added to x on host (GraphNorm is per-channel shift invariant).
- per-slot pipelining: attention(s) -> out_proj(s) -> gnorm2 stats(s); FFN per
  half interleaved with the other half's attention.
"""

import math
import numpy as np
import ml_dtypes

import concourse.bass as bass
import concourse.bacc as bacc
import concourse.mybir as mybir
import concourse.tile as tile
from concourse.bass_utils import run_bass_kernel_spmd
from contextlib import ExitStack

N_CORES = 8
B = 64
H = 256
NH = 8
HD = H // NH
EPS = 1e-5

F32 = mybir.dt.float32
BF16 = mybir.dt.bfloat16
FP8 = mybir.dt.float8e4
AF = mybir.ActivationFunctionType
OP = mybir.AluOpType
PM = mybir.MatmulPerfMode

NEG = -30.0          # additive key mask (pre-exp); exp(-30) == 0 in bf16
SC = 1.0 / math.sqrt(HD)
S1 = 32.0            # ffn_w1 fp8 prescale
S2 = 32.0            # ffn_w2 fp8 prescale
SO = 32.0            # out_proj_w fp8 prescale
SCX = 8.0            # ctx fp8 prescale (via 1/8 in the vr ones-column)


def _plan(batch):
    batch = np.asarray(batch).astype(np.int64)
    counts = np.bincount(batch, minlength=B).astype(np.int64)
    starts = np.concatenate([[0], np.cumsum(counts)[:-1]])
    order = np.argsort(-counts, kind="stable")  # rank -> graph id
    NS = B // N_CORES
    Ms, slot_graph = [], np.zeros((N_CORES, NS), np.int64)
    for s in range(NS):
        blk = order[N_CORES * s: N_CORES * s + N_CORES]
        m = int(max(16, math.ceil(max(1, counts[blk].max()) / 16) * 16))
        Ms.append(m)
        for c in range(N_CORES):
            slot_graph[c, s] = blk[c]
    offs = np.concatenate([[0], np.cumsum(Ms)]).astype(np.int64)
    Rtot = int(offs[-1])
    R = int(math.ceil(Rtot / 128) * 128)
    return counts, starts, slot_graph, Ms, offs, Rtot, R


def _build(nc, Ms, offs, R, ns_valid, pair_gelu):
    """ns_valid[s] = max valid node count over cores for slot s (<= Ms[s]).
    Per-core valid counts differ; we compute the slot at the max width and the
    km mask (per core) zeroes the prob rows beyond each core's own count.
    Query-side trims use ns_valid (same extent every core keeps SPMD single
    program); pads beyond ns_valid are never read back by any core."""
    NS = len(Ms)
    nkt = [math.ceil(m / 128) for m in Ms]
    NKT = sum(nkt)
    MMAX = max(Ms)

    # ---- DRAM ----
    d_xt = nc.dram_tensor("xt", [2, 128, R], BF16, kind="ExternalInput").ap()
    d_wqk = nc.dram_tensor("wqk", [128, 2, 512], BF16, kind="ExternalInput").ap()
    d_wvo = nc.dram_tensor("wvo", [128, 2, 264], BF16, kind="ExternalInput").ap()
    d_wo = nc.dram_tensor("wo", [128, 2, 2, 128], FP8, kind="ExternalInput").ap()
    d_w1 = nc.dram_tensor("w1", [128, 2, 1024], FP8, kind="ExternalInput").ap()
    d_w2 = nc.dram_tensor("w2", [128, 8, 2, 128], FP8, kind="ExternalInput").ap()
    # packed per-partition constants:
    # [qb(2) fb1(8) nb(4) ga1(NS) gAp(4*NS) gBp(4*NS) km(NKT)]
    NCST = 14 + 9 * NS + NKT
    d_cst = nc.dram_tensor("cst", [128, NCST], F32, kind="ExternalInput").ap()
    d_id = nc.dram_tensor("ident", [128, 128], BF16, kind="ExternalInput").ap()
    d_ot = nc.dram_tensor("ot", [2, 128, R], BF16, kind="ExternalOutput").ap()

    mm = nc.tensor.matmul

    with tile.TileContext(nc) as tc, ExitStack() as ctx:
        pers = ctx.enter_context(tc.tile_pool(name="pers", bufs=1))
        ptp = ctx.enter_context(tc.tile_pool(name="ptp", bufs=12))
        hgp = ctx.enter_context(tc.tile_pool(name="hgp", bufs=2))
        stat = ctx.enter_context(tc.tile_pool(name="stat", bufs=4))
        ctxp = ctx.enter_context(tc.tile_pool(name="ctxp", bufs=3))
        psP = ctx.enter_context(tc.tile_pool(name="psP", bufs=2, space="PSUM"))
        psS = ctx.enter_context(tc.tile_pool(name="psS", bufs=2, space="PSUM"))
        psC = ctx.enter_context(tc.tile_pool(name="psC", bufs=2, space="PSUM"))

        # ---- persistent SBUF tiles ----
        cst = pers.tile([128, NCST], F32, name="cst", tag="cst")
        nc.sync.dma_start(out=cst, in_=d_cst)
        co = 0
        def cslice(n):
            nonlocal co
            a = cst[:, co:co + n]; co += n
            return a
        qb = [cslice(1) for _ in range(2)]
        fb1 = [cslice(1) for _ in range(8)]
        nb = [[cslice(1) for _ in range(2)] for _ in range(2)]
        ga1 = cslice(NS)
        gAp = [[cslice(NS) for _ in range(2)] for _ in range(2)]
        gBp = [[cslice(NS) for _ in range(2)] for _ in range(2)]
        km = [cslice(1) for _ in range(NKT)]
        kmi = {}
        idx = 0
        for s in range(NS):
            for kt in range(nkt[s]):
                kmi[(s, kt)] = idx; idx += 1

        # Few LARGE DMAs, critical-first: each DMACopy pays ~1.4us of fixed
        # HWDGE/sem overhead, so slot-granular loads serialize the startup.
        q_off = int(offs[NS // 4])
        half_off = int(offs[NS // 2])
        ident = pers.tile([128, 128], BF16, name="ident", tag="ident")
        nc.sync.dma_start(out=ident, in_=d_id)
        xt = [pers.tile([128, R], BF16, name=f"xt{i}", tag=f"xt{i}") for i in range(2)]
        for ct in range(2):
            nc.sync.dma_start(out=xt[ct][:, 0:q_off], in_=d_xt[ct][:, 0:q_off])
        wqkt = pers.tile([128, 2, 512], BF16, name="wqkt", tag="wqkt")
        nc.sync.dma_start(out=wqkt, in_=d_wqk)
        wvot = pers.tile([128, 2, 264], BF16, name="wvot", tag="wvot")
        nc.sync.dma_start(out=wvot, in_=d_wvo)
        for ct in range(2):
            nc.sync.dma_start(out=xt[ct][:, q_off:half_off], in_=d_xt[ct][:, q_off:half_off])
        for ct in range(2):
            nc.sync.dma_start(out=xt[ct][:, half_off:R], in_=d_xt[ct][:, half_off:R])
        wot = pers.tile([128, 2, 2, 128], FP8, name="wot", tag="wot")
        nc.sync.dma_start(out=wot, in_=d_wo)
        w1 = pers.tile([128, 2, 1024], FP8, name="w1", tag="w1")
        nc.sync.dma_start(out=w1, in_=d_w1)
        w2t = pers.tile([128, 8, 2, 128], FP8, name="w2t", tag="w2t")
        nc.sync.dma_start(out=w2t, in_=d_w2)

        # PE p-state warmup: ~24 dep-free matmuls keep the PE continuously
        # busy from t~1us so real matmuls start at the fast clock.
        for _ in range(36):
            wps = psP.tile([128, 512], F32, name="psp", tag="psp")
            mm(wps[:, :128], ident, ident, start=True, stop=True)

        xn = [pers.tile([128, R], BF16, name=f"xn{i}", tag=f"xn{i}") for i in range(2)]
        qt_ = [pers.tile([128, R], BF16, name=f"q{i}", tag=f"q{i}") for i in range(2)]
        kt_ = [pers.tile([128, R], BF16, name=f"k{i}", tag=f"k{i}") for i in range(2)]
        qZ = pers.tile([128, 8, MMAX], BF16, name="qZ", tag="qZ")
        nc.gpsimd.memset(qZ, 0.0)  # persistent zeros; head h only ever writes rows 32*(h%4)
        vr = pers.tile([128, 264 * NKT], BF16, name="vr", tag="vr")
        ctxt = pers.tile([128, 2, R], FP8, name="ctxt", tag="ctxt")
        x2 = [pers.tile([128, R], BF16, name=f"x2{i}", tag=f"x2{i}") for i in range(2)]
        for ct in range(2):
            nc.gpsimd.memset(x2[ct], 0.0)  # pads must stay 0 for gnorm2 stats
        xn2 = pers.tile([128, 2, R], FP8, name="xn2", tag="xn2")
        out_t = [pers.tile([128, R], BF16, name=f"ot{i}", tag=f"ot{i}") for i in range(2)]

        # ---------- GraphNorm stats+apply ----------
        # rstd = rsqrt(var) via DVE reciprocal seed + Newton (keeps the ACT
        # engine free of Ln/Exp and their 1283ns table loads; eps ~ 1e-5 on a
        # ~1.0 std is far below the fp8/bf16 noise floor, dropped).
        def gnorm_stats(src_f, widx, slots, tg):
            """bn_stats (DVE only) + var/mean prep; returns (var, meanp)."""
            nsl = len(slots)
            c0 = slots[0]
            var = stat.tile([128, 2, nsl], F32, name="var", tag=f"var{tg}")
            meanp = stat.tile([128, 2, nsl], F32, name="meanp", tag=f"meanp{tg}")
            for ct in range(2):
                mv = stat.tile([128, 2, nsl], F32, name="mv", tag=f"mv{tg}{ct}")
                for i, s in enumerate(slots):
                    st6 = stat.tile([128, 6], F32, name="st6", tag="st6")
                    nc.vector.bn_stats(out=st6, in_=src_f(ct)[:, offs[s]:offs[s] + Ms[s]])
                    nc.vector.bn_aggr(out=mv[:, :, i:i + 1], in_=st6)
                mean_r = mv[:, 0:1, :].squeeze(1)
                var_r = mv[:, 1:2, :].squeeze(1)
                m2 = stat.tile([128, nsl], F32, name="m2", tag="m2")
                nc.vector.tensor_mul(m2, mean_r, mean_r)
                v1 = stat.tile([128, nsl], F32, name="v1", tag="v1")
                nc.vector.tensor_mul(v1, var_r, gAp[widx][ct][:, c0:c0 + nsl])
                nc.vector.tensor_mul(var[:, ct, :], m2, gBp[widx][ct][:, c0:c0 + nsl])
                nc.vector.tensor_add(var[:, ct, :], var[:, ct, :], v1)
                nc.vector.tensor_mul(meanp[:, ct, :], mean_r, ga1[:, c0:c0 + nsl])
            return var, meanp

        def gnorm_chain(var, meanp, widx, slots, tg, eng, recip_seed):
            """rsqrt chain -> (y == scale, per-ct shift). Runs on `eng` so the
            ~6us of serial hop latency doesn't head-of-line block DVE."""
            nsl = len(slots)
            vv = var[:, :, :]
            y = stat.tile([128, 2, nsl], F32, name="y", tag=f"y{tg}")
            t = stat.tile([128, 2, nsl], F32, name="t", tag=f"t{tg}")
            if recip_seed:
                nc.vector.reciprocal_approx_fast(out=y, in_=vv)
                eng.tensor_scalar(out=y, in0=y, scalar1=0.5, scalar2=0.5,
                                  op0=OP.mult, op1=OP.add)
                iters = 1
            else:  # linear seed 1.5 - 0.5v (fine for var in [0.5, 1.6])
                eng.tensor_scalar(out=y, in0=vv, scalar1=-0.5, scalar2=1.5,
                                  op0=OP.mult, op1=OP.add)
                iters = 2
            for _ in range(iters):
                eng.tensor_mul(t, vv, y)
                eng.tensor_mul(t, t, y)
                eng.tensor_scalar(out=t, in0=t, scalar1=-0.5, scalar2=1.5,
                                  op0=OP.mult, op1=OP.add)
                eng.tensor_mul(y, y, t)
            sc_sh = []
            for ct in range(2):
                shift = stat.tile([128, nsl], F32, name="shift", tag=f"shift{tg}{ct}")
                eng.tensor_mul(shift, meanp[:, ct, :], y[:, ct, :])
                eng.tensor_scalar(
                    out=shift, in0=shift, scalar1=-1.0, scalar2=nb[widx][ct],
                    op0=OP.mult, op1=OP.add)
                sc_sh.append(shift)
            return y, sc_sh

        def gnorm_apply(src_f, dst_f, y, sc_sh, slots, awidths, eng, sel=None):
            for i, s in enumerate(slots):
                if sel is not None and s not in sel:
                    continue
                for ct in range(2):
                    w = awidths[s]
                    if eng is nc.scalar:  # ACT: out = Identity(scale*in + bias)
                        nc.scalar.activation(
                            out=dst_f(ct, s, w),
                            in_=src_f(ct)[:, offs[s]:offs[s] + w],
                            func=AF.Identity,
                            bias=sc_sh[ct][:, i:i + 1],
                            scale=y[:, ct, i:i + 1])
                    else:
                        eng.tensor_scalar(
                            out=dst_f(ct, s, w),
                            in0=src_f(ct)[:, offs[s]:offs[s] + w],
                            scalar1=y[:, ct, i:i + 1],
                            scalar2=sc_sh[ct][:, i:i + 1],
                            op0=OP.mult, op1=OP.add)

        def gnorm(src_f, dst_f, widx, slots, awidths, tg,
                  chain_eng=None, apply_eng=None, recip_seed=True):
            var, meanp = gnorm_stats(src_f, widx, slots, tg)
            y, sc_sh = gnorm_chain(var, meanp, widx, slots, tg,
                                   chain_eng or nc.vector, recip_seed)
            gnorm_apply(src_f, dst_f, y, sc_sh, slots, awidths,
                        apply_eng or (nc.vector if widx == 0 else nc.gpsimd))
            return y, sc_sh



        # ---------- phase 2: q,k projections (slot-aligned chunks so each
        # slot's qZ depends only on its own gnorm pair; pads never projected) --
        def qkproj_chunk(o, w, on_act=False):
            for mt in range(4):
                ps = psP.tile([128, 512], F32, name="psp", tag="psp")
                for ktc in range(2):
                    mm(ps[:, :w], wqkt[:, ktc, 128 * mt:128 * mt + 128],
                       xn[ktc][:, o:o + w], start=(ktc == 0), stop=(ktc == 1))
                if mt < 2:  # q: add bias (k bias cancels in softmax)
                    if on_act:
                        nc.scalar.activation(out=qt_[mt][:, o:o + w], in_=ps[:, :w],
                                             func=AF.Identity, bias=qb[mt])
                    else:
                        nc.vector.tensor_scalar_add(qt_[mt][:, o:o + w], ps[:, :w], qb[mt])
                else:
                    if on_act:
                        nc.scalar.activation(out=kt_[mt - 2][:, o:o + w], in_=ps[:, :w],
                                             func=AF.Copy)
                    else:
                        nc.vector.tensor_copy(kt_[mt - 2][:, o:o + w], ps[:, :w])

        def qkproj_slot(s, on_act=False):
            qkproj_chunk(int(offs[s]), int(Ms[s]), on_act)

        vri = {}
        idx = 0
        for s in range(NS):
            for kt in range(nkt[s]):
                vri[(s, kt)] = idx; idx += 1
        def vproj(s, kt, on_act=False):
            mkt = min(128, Ms[s] - 128 * kt)
            ko = offs[s] + 128 * kt
            vb = 264 * vri[(s, kt)]
            ps = psP.tile([128, 512], F32, name="psp", tag="psp")
            for ctc in range(2):
                mm(ps[:mkt, :264], xn[ctc][:, ko:ko + mkt], wvot[:, ctc],
                   start=(ctc == 0), stop=(ctc == 1))
            if on_act:
                nc.scalar.activation(out=vr[:mkt, vb:vb + 264], in_=ps[:mkt, :264],
                                     func=AF.Copy)
            else:
                nc.vector.tensor_copy(vr[:mkt, vb:vb + 264], ps[:mkt, :264])
            # sumexp ones-column = 1/SCX (ctx fp8 prescale rides the ratio)
            ones = vr[:mkt, vb:vb + 264].rearrange("p (h c) -> p h c", h=8, c=33)[:, :, 32:33].squeeze(2)
            nc.gpsimd.memset(ones, 1.0 / SCX)

        # ---------- attention, software-pipelined over (slot, qtile) ----------
        def qz_slot(s, on_dve):
            M = Ms[s]
            eng = nc.vector if on_dve else nc.gpsimd
            for h in range(8):
                hp = 32 * (h % 4)
                eng.tensor_copy(qZ[hp:hp + 32, h, :M],
                                qt_[h // 4][hp:hp + 32, offs[s]:offs[s] + M])

        def attn_front(s, qi):
            """scores + exp for all key tiles of (s, qi); returns state."""
            M = Ms[s]; nv = ns_valid[s]
            qo = 128 * qi
            qc = min(128, nv - qo)
            pts = []
            for kt in range(nkt[s]):
                mkt = min(128, M - 128 * kt)
                ko = offs[s] + 128 * kt
                st = psS.tile([128, 1024], F32, name="st", tag="st")
                for h in range(8):
                    mm(st[:mkt, 128 * h:128 * h + qc],
                       kt_[h // 4][:, ko:ko + mkt], qZ[:, h, qo:qo + qc],
                       start=True, stop=True)
                pt = ptp.tile([128, 1024], BF16, name="pt", tag="pt")
                stv = st[:mkt, :].rearrange("p (h c) -> p h c", h=8, c=128)[:, :, :qc]
                ptv = pt[:mkt, :].rearrange("p (h c) -> p h c", h=8, c=128)[:, :, :qc]
                nc.scalar.activation(out=ptv, in_=stv, func=AF.Exp,
                                     bias=km[kmi[(s, kt)]][:mkt], scale=SC)
                pts.append(pt)
            return pts

        def attn_back(s, qi, pts):
            """PV + divide + transpose + ctxt store for (s, qi)."""
            M = Ms[s]; nv = ns_valid[s]
            qo = 128 * qi
            qc = min(128, nv - qo)
            qbase = offs[s] + qo
            # cs ([128,264] f32) + tp ([128,2,128] bf16 via bitcast) share
            # one PSUM bank: 1056B + 512B < 2KB
            csbank = psC.tile([128, 392], F32, name="csbank", tag="cs")
            cs = csbank[:, 0:264]
            tp = csbank[:, 264:392].bitcast(BF16).rearrange(
                "p (t c) -> p t c", t=2, c=128)
            for kt in range(nkt[s]):
                mkt = min(128, M - 128 * kt)
                vb = 264 * vri[(s, kt)]
                last = kt == nkt[s] - 1
                for h in range(8):
                    mm(cs[:qc, 33 * h:33 * h + 33],
                       pts[kt][:mkt, 128 * h:128 * h + qc],
                       vr[:mkt, vb + 33 * h:vb + 33 * h + 33],
                       start=(kt == 0), stop=last)
            # rec = SCX / sumexp ; ctxT = cs * rec (broadcast over 33-blocks)
            rec = stat.tile([128, 8], F32, name="rec", tag="rec")
            den = cs[:qc, :].rearrange("p (h c) -> p h c", h=8, c=33)[:, :, 32:33].squeeze(2)
            nc.vector.reciprocal_approx_fast(out=rec[:qc, :], in_=den)
            ctxTs = ctxp.tile([128, 256], BF16, name="ctxTs", tag="ctxTs")
            csv = cs[:qc, :].rearrange("p (h c) -> p h c", h=8, c=33)[:, :, 0:32]
            ctv = ctxTs[:qc, :].rearrange("p (h c) -> p h c", h=8, c=32)
            rv = rec[:qc, :].unsqueeze(2).broadcast_to([qc, 8, 32])
            nc.vector.tensor_tensor(out=ctv, in0=csv, in1=rv, op=OP.mult)
            for ctc in range(2):
                nc.tensor.transpose(tp[:, ctc, :qc], ctxTs[:qc, 128 * ctc:128 * ctc + 128],
                                    ident[:qc, :qc])
            nc.vector.tensor_copy(ctxt[:, :, qbase:qbase + qc], tp[:, :, :qc])

        # ---------- out_proj + residual (valid width only; pads stay 0) ----
        def outproj_slot(s):
            nv = ns_valid[s]
            o = offs[s]
            for ctc in range(2):
                ps = psP.tile([128, 512], F32, name="psp", tag="psp")
                for cw in range(0, nv, 256):  # DoubleRow rhs free = 2*w <= 512
                    w = min(256, nv - cw)
                    mm(ps[:, cw:cw + w], wot[:, ctc], ctxt[:, :, o + cw:o + cw + w],
                       start=True, stop=True, perf_mode=PM.DoubleRow)
                nc.vector.scalar_tensor_tensor(
                    out=x2[ctc][:, o:o + nv], in0=ps[:, :nv], scalar=1.0 / (SCX * SO),
                    in1=xt[ctc][:, o:o + nv], op0=OP.mult, op1=OP.add)

        # ---------- FFN, two stages for slot-level staggering ----------
        def ffn1_slot(s, pair_gelu):
            nv = ns_valid[s]
            o = offs[s]
            hg = [hgp.tile([128, 2, MMAX], FP8, name=f"hg{p}", tag=f"hg{p}") for p in range(4)]
            for p in range(4):
                if pair_gelu:
                    # one [128,1024] psum holds the mt-pair; one Gelu covers both
                    ps = psS.tile([128, 1024], F32, name="st", tag="st")
                    for half in range(2):
                        mt = 2 * p + half
                        for cw in range(0, nv, 256):
                            w = min(256, nv - cw)
                            mm(ps[:, 512 * half + cw:512 * half + cw + w],
                               w1[:, :, 128 * mt:128 * mt + 128],
                               xn2[:, :, o + cw:o + cw + w],
                               start=True, stop=True, perf_mode=PM.DoubleRow)
                    psv = ps[:, :].rearrange("p (t c) -> p t c", t=2, c=512)[:, :, :nv]
                    nc.scalar.activation(out=hg[p][:, :, :nv], in_=psv,
                                         func=AF.Gelu, scale=1.0 / S1)
                else:
                    for half in range(2):
                        mt = 2 * p + half
                        ps = psP.tile([128, 512], F32, name="psp", tag="psp")
                        for cw in range(0, nv, 256):
                            w = min(256, nv - cw)
                            mm(ps[:, cw:cw + w], w1[:, :, 128 * mt:128 * mt + 128],
                               xn2[:, :, o + cw:o + cw + w],
                               start=True, stop=True, perf_mode=PM.DoubleRow)
                        nc.scalar.activation(out=hg[p][:, half, :nv], in_=ps[:, :nv],
                                             func=AF.Gelu, bias=fb1[mt], scale=1.0 / S1)
            return hg

        def ffn2_slot(s, hg):
            nv = ns_valid[s]
            o = offs[s]
            for ctc in range(2):
                ps2 = psP.tile([128, 512], F32, name="psp", tag="psp")
                for cw in range(0, nv, 256):
                    w = min(256, nv - cw)
                    for p in range(4):
                        mm(ps2[:, cw:cw + w], w2t[:, 4 * ctc + p], hg[p][:, :, cw:cw + w],
                           start=(p == 0), stop=(p == 3), perf_mode=PM.DoubleRow)
                nc.vector.scalar_tensor_tensor(
                    out=out_t[ctc][:, o:o + nv], in0=ps2[:, :nv], scalar=1.0 / S2,
                    in1=x2[ctc][:, o:o + nv], op0=OP.mult, op1=OP.add)

        # ---------- main pipelined emission ----------
        # attention staggered by one (s, qi) item: scores/exp of item i+1 sit
        # ahead of PV(i) in the PE queue, so PE never head-of-line blocks on
        # an exp. FFN staggered by one slot for the same reason. ACT stream
        # stays [exps..., gelus...] to avoid 1283ns table reloads.
        # startup: absolute minimum before the first scores/exp of slot 0 —
        # gnorm1 pair {s0,s1}, projections of s0/s1, qZ(0), vproj(0). The
        # other pairs are emitted inside the first loop iterations and
        # pipeline their (stats -> rsqrt chain -> apply -> proj) latency
        # behind the running exp stream.
        xn_dst = lambda ct, s, w: xn[ct][:, offs[s]:offs[s] + w]
        xt_src = lambda ct: xt[ct]
        MW = {s: Ms[s] for s in range(NS)}
        # pair 0 start path: stats(DVE) -> chain(Pool) -> applies+proj(ACT) ->
        # qZ(DVE). The ACT engine is idle until the first exp, and the Pool
        # chain keeps DVE free for the stats of pairs 1-3 (DMA-gated).
        v0, m0 = gnorm_stats(xt_src, 0, [0, 1], "g1p0")
        y0_, s0_ = gnorm_chain(v0, m0, 0, [0, 1], "g1p0", nc.gpsimd, recip_seed=False)
        gnorm_apply(xt_src, xn_dst, y0_, s0_, [0, 1], MW, nc.scalar)
        qkproj_slot(0, on_act=True)
        qz_slot(0, True)
        qkproj_slot(1, on_act=True)
        for kt in range(nkt[0]):
            vproj(0, kt, on_act=True)
        qz_slot(1, True)
        for kt in range(nkt[1]):
            vproj(1, kt, on_act=True)
        g1ys = {0: (y0_, s0_)}
        for p in (1, 2, 3):
            vp, mp = gnorm_stats(xt_src, 0, [2 * p, 2 * p + 1], f"g1p{p}")
            g1ys[p] = gnorm_chain(vp, mp, 0, [2 * p, 2 * p + 1], f"g1p{p}",
                                  nc.gpsimd, recip_seed=False)

        items = [(s, qi) for s in range(NS) for qi in range(math.ceil(ns_valid[s] / 128))]
        DEPTH = 2
        fronts = {}

        def do_back(j):
            sj, qj = items[j]
            attn_back(sj, qj, fronts.pop(j))
            if qj == math.ceil(ns_valid[sj] / 128) - 1:
                outproj_slot(sj)
                if sj == 3:
                    gnorm(lambda ct: x2[ct],
                          lambda ct, s2, w: xn2[:, ct, offs[s2]:offs[s2] + w],
                          1, [0, 1, 2, 3], {t: ns_valid[t] for t in range(NS)}, "g2a",
                          chain_eng=nc.gpsimd, recip_seed=False)

        for i, (s, qi) in enumerate(items):
            nq_s = math.ceil(ns_valid[s] / 128)
            if qi == nq_s - 1 and s <= 2:
                # pair s+1 applies + projections on ACT at the slot boundary
                p = s + 1
                yp, sp = g1ys[p]
                gnorm_apply(xt_src, xn_dst, yp, sp, [2 * p, 2 * p + 1], MW,
                            nc.vector)
                qkproj_slot(2 * p)
                qkproj_slot(2 * p + 1)
            if qi == 0 and 1 <= s and s + 1 < NS:
                # one slot of lead: slot s+1 qZ + vr on DVE (ready by now)
                qz_slot(s + 1, True)
                for kt in range(nkt[s + 1]):
                    vproj(s + 1, kt)
            fronts[i] = attn_front(s, qi)
            if i >= DEPTH:
                do_back(i - DEPTH)
        for j in range(len(items) - DEPTH, len(items)):
            do_back(j)
        gnorm(lambda ct: x2[ct],
              lambda ct, s2, w: xn2[:, ct, offs[s2]:offs[s2] + w],
              1, [4, 5, 6, 7], {t: ns_valid[t] for t in range(NS)}, "g2b",
              chain_eng=nc.gpsimd, recip_seed=False)
        hgprev = None
        for s in range(NS):
            hgnew = ffn1_slot(s, pair_gelu)
            if hgprev is not None:
                ffn2_slot(s - 1, hgprev)
                if s % 2 == 0 and s >= 2:  # pair (s-2, s-1) fully stored
                    for ctc in range(2):
                        nc.sync.dma_start(
                            out=d_ot[ctc][:, offs[s - 2]:offs[s]],
                            in_=out_t[ctc][:, offs[s - 2]:offs[s]])
            hgprev = hgnew
        ffn2_slot(NS - 1, hgprev)
        for ctc in range(2):
            nc.sync.dma_start(out=d_ot[ctc][:, offs[NS - 2]:R],
                              in_=out_t[ctc][:, offs[NS - 2]:R])
    return nc


_CACHE = {}


def _prepare(inputs):
    x = np.asarray(inputs["x"], np.float32)
    batch = np.asarray(inputs["batch"]).astype(np.int64)
    counts, starts, slot_graph, Ms, offs, Rtot, R = _plan(batch)
    NS = len(Ms)
    nkt = [math.ceil(m / 128) for m in Ms]
    NKT = sum(nkt)

    in_proj_w = np.asarray(inputs["in_proj_w"], np.float32)
    in_proj_b = np.asarray(inputs["in_proj_b"], np.float32)
    out_proj_w = np.asarray(inputs["out_proj_w"], np.float32)
    out_proj_b = np.asarray(inputs["out_proj_b"], np.float32)
    ffn_w1 = np.asarray(inputs["ffn_w1"], np.float32)
    ffn_b1 = np.asarray(inputs["ffn_b1"], np.float32)
    ffn_w2 = np.asarray(inputs["ffn_w2"], np.float32)
    ffn_b2 = np.asarray(inputs["ffn_b2"], np.float32)

    # biases folded into the residual stream (gnorm is shift-invariant):
    # x' = x + out_proj_b + wo @ v_bias + ffn_b2
    fold = out_proj_b + out_proj_w @ in_proj_b[2 * H:3 * H] + ffn_b2
    xb = x + fold[None, :]

    wqk = np.ascontiguousarray(
        in_proj_w[:2 * H].T.reshape(2, 128, 512).transpose(1, 0, 2)).astype(ml_dtypes.bfloat16)
    # wv expanded to 33-col stride with zero ones-columns
    wvT = in_proj_w[2 * H:].T.reshape(2, 128, 8, 32)
    wvo = np.zeros((2, 128, 8, 33), np.float32)
    wvo[:, :, :, :32] = wvT
    wvo = np.ascontiguousarray(
        wvo.reshape(2, 128, 264).transpose(1, 0, 2)).astype(ml_dtypes.bfloat16)
    # wo fp8 [ct_out][128, 2(plane=ct_in), 128], prescaled
    woT = (out_proj_w.T * SO).reshape(2, 128, 2, 128)   # [ct_in, part, ct_out, col]
    wo8 = np.ascontiguousarray(woT.transpose(1, 2, 0, 3)).astype(ml_dtypes.float8_e4m3)
    # w1 fp8 [128, 2, 1024]: plane = input ct
    w18 = np.ascontiguousarray((ffn_w1.T * S1).reshape(2, 128, 1024).transpose(1, 0, 2)).astype(ml_dtypes.float8_e4m3)
    # w2 fp8 [8][128, 2, 128]: idx = 4*ct_out + pair; plane i = hidden 256p+128i
    w2T = (ffn_w2.T * S2).reshape(4, 2, 128, 2, 128)    # [pair, plane, part, ct_out, col]
    # [part, idx=4*ct_out+pair, plane, col]
    w28 = np.ascontiguousarray(
        w2T.transpose(2, 3, 0, 1, 4).reshape(128, 8, 2, 128)).astype(ml_dtypes.float8_e4m3)

    qkb = in_proj_b[:2 * H].reshape(4, 128)
    fb1 = ffn_b1.reshape(8, 128)
    nwv = np.stack([np.asarray(inputs["norm1_w"], np.float32).reshape(2, 128),
                    np.asarray(inputs["norm2_w"], np.float32).reshape(2, 128)])
    nbv = np.stack([np.asarray(inputs["norm1_b"], np.float32).reshape(2, 128),
                    np.asarray(inputs["norm2_b"], np.float32).reshape(2, 128)])

    xT = xb.T  # [256, N]
    xts = np.zeros((N_CORES, 2, 128, R), np.float32)
    ga1 = np.zeros((N_CORES, 128, NS), np.float32)
    gA = np.zeros((N_CORES, 128, NS), np.float32)
    gB = np.zeros((N_CORES, 128, NS), np.float32)
    kms = np.full((N_CORES, NKT, 128), NEG, np.float32)
    ns_valid = [0] * NS
    for c in range(N_CORES):
        for s in range(NS):
            g = slot_graph[c, s]
            n = int(counts[g])
            st = int(starts[g])
            o = int(offs[s])
            ns_valid[s] = max(ns_valid[s], n)
            if n > 0:
                blk = xT[:, st:st + n]
                xts[c, 0, :, o:o + n] = blk[:128]
                xts[c, 1, :, o:o + n] = blk[128:]
            ne = max(n, 1)
            ga1[c, :, s] = Ms[s] / ne
            inv_nm1 = 1.0 / max(ne - 1, 1)
            gA[c, :, s] = Ms[s] * inv_nm1
            gB[c, :, s] = Ms[s] * (1.0 - Ms[s] / ne) * inv_nm1
            ki = sum(nkt[:s])
            for kt in range(nkt[s]):
                v = min(128, max(0, n - 128 * kt))
                kms[c, ki + kt, :v] = 0.0
    ns_valid = [int(math.ceil(v / 16) * 16) if v % 16 else v for v in ns_valid]
    ns_valid = [min(v, Ms[s]) for s, v in enumerate(ns_valid)]

    pair_gelu = bool(np.all(ffn_b1 == 0))
    key = (tuple(Ms), R, tuple(ns_valid), pair_gelu)
    if key not in _CACHE:
        nc = bacc.Bacc("TRN2", target_bir_lowering=False, debug=False,
                       num_devices=N_CORES)
        _build(nc, Ms, offs, R, ns_valid, pair_gelu)
        nc.compile()
        _CACHE[key] = nc
    nc = _CACHE[key]

    ident = np.eye(128).astype(ml_dtypes.bfloat16)
    in_maps = []
    for c in range(N_CORES):
        # norm_w folded: y = rsqrt(var / nw^2) = nw * rsqrt(var) (nw >= 0)
        nw2 = np.maximum(nwv.reshape(2, 2, 128), 1e-20) ** 2  # [phase, ct, 128]
        gparts = []
        for gsrc in (gA[c], gB[c]):           # [128, NS]
            for ph in range(2):
                for ctc in range(2):
                    gparts.append(gsrc / nw2[ph, ctc][:, None])
        cstc = np.concatenate(
            [qkb[:2].T,                       # qb (q bias only)
             fb1.T,                           # 8
             nbv.reshape(4, 128).T,           # 4
             ga1[c]] + gparts +               # NS + 8*NS
            [kms[c].T], axis=1).astype(np.float32)
        in_maps.append({
            "xt": xts[c].astype(ml_dtypes.bfloat16),
            "wqk": wqk, "wvo": wvo, "wo": wo8, "w1": w18, "w2": w28,
            "cst": np.ascontiguousarray(cstc),
            "ident": ident,
        })

    def unpack(outs):
        out = np.empty((x.shape[0], H), np.float32)
        for c in range(N_CORES):
            ot = np.asarray(outs[c]["ot"]).astype(np.float32)  # [2, 128, R]
            full = np.concatenate([ot[0], ot[1]], axis=0)      # [256, R]
            for s in range(NS):
                g = slot_graph[c, s]
                n = int(counts[g])
                st = int(starts[g])
                o = int(offs[s])
                if n > 0:
                    out[st:st + n] = full[:, o:o + n].T
        return out

    return nc, in_maps, unpack


def kernel(**inputs):
    nc, in_maps, unpack = _prepare(inputs)
    res = run_bass_kernel_spmd(nc, in_maps, list(range(N_CORES)))
    return unpack(res.results)


def _traced_run(**inputs):
    """Cost-model timeline (single core) + warm wall-clock. Returns model ns."""
    import time
    nc, in_maps, unpack = _prepare(inputs)
    t0 = time.time()
    run_bass_kernel_spmd(nc, in_maps, list(range(N_CORES)))
    t1 = time.time()
    run_bass_kernel_spmd(nc, in_maps, list(range(N_CORES)))
    t2 = time.time()
    print(f"wall cold: {t1 - t0:.2f}s  warm: {t2 - t1:.2f}s")
    from concourse.timeline_sim import TimelineSim
    import trails.perfetto as _tp
    for _m in ("enable_explicit_ordering", "reserve_process_order",
               "reserve_thread_order", "set_process_order", "set_thread_order",
               "add_instant"):
        if not hasattr(_tp.LazyPerfetto, _m):
            setattr(_tp.LazyPerfetto, _m, lambda self, *a, **k: None)
    if not hasattr(_tp.LazyPerfetto, "add_counter"):
        def _add_counter(self, *a, **k):
            try:
                self.update_counter(*a, **k)
            except Exception:
                pass
        _tp.LazyPerfetto.add_counter = _add_counter
    tl = TimelineSim(nc, trace=True)
    total = tl.simulate()
    pf = tl.perfetto
    if callable(pf):
        pf = pf()
    if pf is not None:
        try:
            pf.save("/root/problem/tl.perfetto-trace")
        except Exception as e:
            print("perfetto dump failed:", e)
    return total
